# revision 22
# baseline (speedup 1.0000x reference)
"""Trainium2 Bass kernel for nn_AttentionGraphEncoder (gnn_message_passing).

Math restructure (exact, not approximate):
  The reference computes, per batch b:
    nf[n]   = embed(node_feats[b,n])                  # [N, E] affine in x
    k[n]    = curr_emb @ Wk1 + nf[n] @ Wk2
    logits  = NORM * q . k[n]  -> softmax (masked)
    v[n]    = curr_emb @ Wv1 + nf[n] @ Wv2
    h       = sum_n attn[n] v[n]
  Since q.k[n] = const + nf[n].(Wk2 @ q) and nf[n] is affine in the raw
  3-dim node coordinates, the logits reduce to x[n] . w3(b) + const with
  w3(b) = W_node @ Wk2 @ q(b)  (a 3-vector per batch).  Likewise
    h = curr_emb @ Wv1 + (a0*depot_emb + (1-a0)*b_node + s3 @ W_node) @ Wv2
  with s3 = sum_{n>=1} attn[n] x[n] (a 3-vector) and a0 = attn[0].
  So the big [N,2E]@[2E,E] matmuls disappear; the kernel streams
  node_feats ([B,N,3]) once through the DVE and does tiny matmuls.

Sharding: pure data parallel, batch 256 -> 32 per core across 8 cores.

Sync discipline: this toolchain's codegen allows at most ONE sync-wait
per instruction.  Same-engine deps are free (in-order queues), so:
  - PE consumes only DVE-written SBUF tiles or gpsimd constants that an
    early PE warm-up op already synced on;
  - every DVE op reads at most one non-DVE source (DMA'd tiles are
    funneled through a DVE copy, wide streams are consumed in chunks
    matching their per-chunk DMA).
"""

import math

import numpy as np

B, N, NODE_DIM, STATE_DIM, EMB = 256, 2048, 3, 4, 128
NCORES = 8
BL = B // NCORES          # 32 batch elements per core
J = 4                     # node-chunks per batch -> 128 partitions (j*BL + b)
NF = N // J               # 512 nodes per partition row
KC = 4                    # free-dim DMA/compute chunks of the x stream
NFC = NF // KC            # 128 nodes per chunk
NORM = 1.0 / math.sqrt(EMB)
MASK_BIG = 400.0          # pre-NORM additive mask magnitude (400*NORM ~ 35)

_CACHE = {}


def _build(finalize=True):
    import concourse.bacc as bacc
    import concourse.bass as bass
    import concourse.mybir as mybir
    import concourse.tile as tile
    from concourse.masks import make_identity

    fp32 = mybir.dt.float32
    i32 = mybir.dt.int32
    u8 = mybir.dt.uint8
    Alu = mybir.AluOpType
    Act = mybir.ActivationFunctionType
    X = mybir.AxisListType.X

    nc = bacc.Bacc("TRN2")

    nfd = nc.dram_tensor("node_feats", [BL, N, NODE_DIM], fp32, kind="ExternalInput")
    std = nc.dram_tensor("state", [BL, STATE_DIM], fp32, kind="ExternalInput")
    Wn = nc.dram_tensor("W_node", [NODE_DIM, EMB], fp32, kind="ExternalInput")
    bn = nc.dram_tensor("b_node", [EMB], fp32, kind="ExternalInput")
    Wd = nc.dram_tensor("W_depot", [2, EMB], fp32, kind="ExternalInput")
    bd = nc.dram_tensor("b_depot", [EMB], fp32, kind="ExternalInput")
    Ws = nc.dram_tensor("W_state", [STATE_DIM, EMB], fp32, kind="ExternalInput")
    bs = nc.dram_tensor("b_state", [EMB], fp32, kind="ExternalInput")
    w7s = nc.dram_tensor("w7s", [7, EMB], fp32, kind="ExternalInput")
    wq = nc.dram_tensor("w_q", [3 * EMB, EMB], fp32, kind="ExternalInput")
    wk2 = nc.dram_tensor("wk2", [EMB, EMB], fp32, kind="ExternalInput")
    wv = nc.dram_tensor("w_v", [2 * EMB, EMB], fp32, kind="ExternalInput")
    ids = nc.dram_tensor("ids", [BL, 2], i32, kind="ExternalInput")
    mk = nc.dram_tensor("mask_u8", [BL, N], u8, kind="ExternalInput")
    out = nc.dram_tensor("out", [BL, EMB], fp32, kind="ExternalOutput")

    with tile.TileContext(nc, pool_alloc_mode="queue") as tc:
        with (
            tc.tile_pool(name="sb", bufs=1) as sb,
            tc.tile_pool(name="ps", bufs=3, space="PSUM") as ps,
            tc.tile_pool(name="pse", bufs=1, space="PSUM") as pse,
            tc.tile_pool(name="psq", bufs=2, space="PSUM") as psq,
        ):
            # ------- gpsimd constants (all before the PE warm-up) -------
            ident = sb.tile([128, 128], fp32)
            make_identity(nc, ident[:])
            ones1 = sb.tile([1, 128], fp32)
            nc.gpsimd.memset(ones1[:], 1.0)
            iota_p = sb.tile([BL, 1], i32)
            nc.gpsimd.iota(iota_p[:], pattern=[[0, 1]], base=0,
                           channel_multiplier=N)
            # rep_eye[p, y] = 1 iff p % BL == y  (cross-j reduce as a matmul)
            rep_eye = sb.tile([128, BL], fp32)
            nc.gpsimd.memset(rep_eye[:], 0.0)
            for j in range(J):
                nc.gpsimd.affine_select(
                    out=rep_eye[:], in_=rep_eye[:],
                    compare_op=Alu.not_equal, fill=1.0,
                    base=-BL * j, pattern=[[-1, BL]], channel_multiplier=1)
            # PE warm-up: one op depending on the LAST gpsimd constant so
            # later PE ops see all Pool ticks as observed.
            junk_p = ps.tile([1, 1], fp32, tag="pt")
            nc.tensor.matmul(junk_p[:], lhsT=rep_eye[:, 0:1],
                             rhs=ident[:, 0:1], start=True, stop=True)

            # ------------------- input DMAs -------------------
            wq_sb = sb.tile([128, 3 * EMB], fp32)
            for t in range(3):
                nc.sync.dma_start(wq_sb[:, t * EMB:(t + 1) * EMB],
                                  wq[t * EMB:(t + 1) * EMB, :])
            wk2_sb = sb.tile([128, EMB], fp32)
            nc.sync.dma_start(wk2_sb[:], wk2[:])
            wv1_sb = sb.tile([128, EMB], fp32)
            nc.sync.dma_start(wv1_sb[:], wv[0:EMB, :])
            wv2_sb = sb.tile([128, EMB], fp32)
            nc.sync.dma_start(wv2_sb[:], wv[EMB:2 * EMB, :])
            w7 = sb.tile([7, 128], fp32)
            nc.sync.dma_start(w7[:], w7s[:])
            ws_sb = sb.tile([STATE_DIM, 128], fp32)
            nc.sync.dma_start(ws_sb[:], Ws[:])
            wn3 = sb.tile([NODE_DIM, 128], fp32)
            nc.sync.dma_start(wn3[:], Wn[:])
            wd2 = sb.tile([2, 128], fp32)
            nc.sync.dma_start(wd2[:], Wd[:])
            bn_c = sb.tile([128, 1], fp32)
            nc.sync.dma_start(bn_c[:], bn[:, None])
            bd_c = sb.tile([128, 1], fp32)
            nc.sync.dma_start(bd_c[:], bd[:, None])
            bs_c = sb.tile([128, 1], fp32)
            nc.sync.dma_start(bs_c[:], bs[:, None])
            ids_sb = sb.tile([BL, 2], i32)
            nc.sync.dma_start(ids_sb[:], ids[:])
            st4 = sb.tile([BL, STATE_DIM], fp32)
            nc.sync.dma_start(st4[:], std[:])
            x03 = sb.tile([BL, NODE_DIM], fp32)
            nc.sync.dma_start(x03[:], nfd[:, 0, :])

            # node_feats stream: [(j b), (f c)] with per-chunk DMAs along free.
            # high_priority: schedule these big DMAs first (they are the bulk
            # of the memory traffic) and pin their SBUF zones before any
            # small-tile churn.
            x = sb.tile([128, NF * 3], fp32)
            mku = sb.tile([128, NF], u8)
            with tc.high_priority():
                xview = (nfd[:].rearrange("b (j f) c -> b j (f c)", j=J)
                         .transpose([1, 0, 2]))             # [J, BL, NF*3]
                for k in range(KC):
                    ksl = slice(k * NFC * 3, (k + 1) * NFC * 3)
                    nc.sync.dma_start(x[:, ksl], xview[:, :, ksl])
                mview = (mk[:].rearrange("b (j f) -> b j f", j=J)
                         .transpose([1, 0, 2]))             # [J, BL, NF]
                for k in range(KC):
                    ksl = slice(k * NFC, (k + 1) * NFC)
                    nc.sync.dma_start(mku[:, ksl], mview[:, :, ksl])

            # ------------- DVE funnels (1 DMA dep each) -------------
            wq_c = sb.tile([128, 3 * EMB], fp32)
            for t in range(3):
                tsl = slice(t * EMB, (t + 1) * EMB)
                nc.vector.tensor_copy(wq_c[:, tsl], wq_sb[:, tsl])
            wk2_c = sb.tile([128, EMB], fp32)
            nc.vector.tensor_copy(wk2_c[:], wk2_sb[:])
            wv1_c = sb.tile([128, EMB], fp32)
            nc.vector.tensor_copy(wv1_c[:], wv1_sb[:])
            wv2_c = sb.tile([128, EMB], fp32)
            nc.vector.tensor_copy(wv2_c[:], wv2_sb[:])
            w7c = sb.tile([7, 128], fp32)
            nc.vector.tensor_copy(w7c[:], w7[:])
            ws_c = sb.tile([STATE_DIM, 128], fp32)
            nc.vector.tensor_copy(ws_c[:], ws_sb[:])
            wn3_c = sb.tile([NODE_DIM, 128], fp32)
            nc.vector.tensor_copy(wn3_c[:], wn3[:])
            wd2_c = sb.tile([2, 128], fp32)
            nc.vector.tensor_copy(wd2_c[:], wd2[:])
            bn_cc = sb.tile([128, 1], fp32)
            nc.vector.tensor_copy(bn_cc[:], bn_c[:])
            bd_cc = sb.tile([128, 1], fp32)
            nc.vector.tensor_copy(bd_cc[:], bd_c[:])
            bs_cc = sb.tile([128, 1], fp32)
            nc.vector.tensor_copy(bs_cc[:], bs_c[:])
            ids_c = sb.tile([BL, 2], i32)
            nc.vector.tensor_copy(ids_c[:], ids_sb[:])
            st_c = sb.tile([BL, STATE_DIM], fp32)
            nc.vector.tensor_copy(st_c[:], st4[:])
            x0_c = sb.tile([BL, NODE_DIM], fp32)
            nc.vector.tensor_copy(x0_c[:], x03[:])

            # ---------------- gathers ----------------
            offs = sb.tile([BL, 2], i32)
            nc.vector.tensor_tensor(offs[:], ids_c[:],
                                    iota_p[:].to_broadcast([BL, 2]),
                                    op=Alu.add)
            nfr = nfd[:].rearrange("b n c -> (b n) c")
            xc3 = sb.tile([BL, NODE_DIM], fp32)
            nc.gpsimd.indirect_dma_start(
                out=xc3[:], out_offset=None, in_=nfr,
                in_offset=bass.IndirectOffsetOnAxis(ap=offs[:, 0:1], axis=0))
            xn3 = sb.tile([BL, NODE_DIM], fp32)
            nc.gpsimd.indirect_dma_start(
                out=xn3[:], out_offset=None, in_=nfr,
                in_offset=bass.IndirectOffsetOnAxis(ap=offs[:, 1:2], axis=0))
            xc_c = sb.tile([BL, NODE_DIM], fp32)
            nc.vector.tensor_copy(xc_c[:], xc3[:])
            xn_c = sb.tile([BL, NODE_DIM], fp32)
            nc.vector.tensor_copy(xn_c[:], xn3[:])
            isz2 = sb.tile([BL, 2], fp32)
            nc.vector.tensor_scalar(isz2[:], ids_c[:], 0, None,
                                    op0=Alu.is_equal)

            # ------------- transposes to [feat, b] -------------
            idb = ident[0:BL, 0:BL]
            stT_p = ps.tile([STATE_DIM, BL], fp32, tag="pt")
            nc.tensor.transpose(stT_p[:], st_c[:], idb)
            stT = sb.tile([STATE_DIM, BL], fp32)
            nc.vector.tensor_copy(stT[:], stT_p[:])
            xcT_p = ps.tile([NODE_DIM, BL], fp32, tag="pt")
            nc.tensor.transpose(xcT_p[:], xc_c[:], idb)
            xcT = sb.tile([NODE_DIM, BL], fp32)
            nc.vector.tensor_copy(xcT[:], xcT_p[:])
            xnT_p = ps.tile([NODE_DIM, BL], fp32, tag="pt")
            nc.tensor.transpose(xnT_p[:], xn_c[:], idb)
            xnT = sb.tile([NODE_DIM, BL], fp32)
            nc.vector.tensor_copy(xnT[:], xnT_p[:])
            x0T_p = ps.tile([NODE_DIM, BL], fp32, tag="pt")
            nc.tensor.transpose(x0T_p[:], x0_c[:], idb)
            x0T = sb.tile([NODE_DIM, BL], fp32)
            nc.vector.tensor_copy(x0T[:], x0T_p[:])
            iszT_p = ps.tile([1, 2 * BL], fp32, tag="pt")
            nc.tensor.transpose(iszT_p[0:1, 0:BL], isz2[:, 0:1], idb)
            nc.tensor.transpose(iszT_p[0:1, BL:2 * BL], isz2[:, 1:2], idb)
            iszT = sb.tile([1, 2 * BL], fp32)
            nc.vector.tensor_copy(iszT[:], iszT_p[:])
            wk2T_p = ps.tile([128, 128], fp32, tag="pt")
            nc.tensor.transpose(wk2T_p[:], wk2_c[:], ident[:])
            wk2T = sb.tile([128, 128], fp32)
            nc.vector.tensor_copy(wk2T[:], wk2T_p[:])
            w7T_p = ps.tile([128, 7], fp32, tag="pt")
            nc.tensor.transpose(w7T_p[:], w7c[:], ident[0:7, 0:7])
            w7T = sb.tile([128, 7], fp32)
            nc.vector.tensor_copy(w7T[:], w7T_p[:])

            # ---------- embeddings (transposed [E, b]) ----------
            # one PSUM tile, 4 col-blocks: se | curr | next | depot
            pemb = pse.tile([128, 4 * BL], fp32, tag="pemb")
            nc.tensor.matmul(pemb[:, 0:BL], lhsT=ws_c[:], rhs=stT[:],
                             start=True, stop=True)
            nc.tensor.matmul(pemb[:, BL:2 * BL], lhsT=wn3_c[:], rhs=xcT[:],
                             start=True, stop=True)
            nc.tensor.matmul(pemb[:, 2 * BL:3 * BL], lhsT=wn3_c[:], rhs=xnT[:],
                             start=True, stop=True)
            nc.tensor.matmul(pemb[:, 3 * BL:4 * BL], lhsT=wd2_c[:],
                             rhs=x0T[0:2, :], start=True, stop=True)
            stateT = sb.tile([128, BL], fp32)
            nc.vector.tensor_scalar(stateT[:], pemb[:, 0:BL], bs_cc[:], None,
                                    op0=Alu.add)
            currN = sb.tile([128, BL], fp32)
            nc.vector.tensor_scalar(currN[:], pemb[:, BL:2 * BL], bn_cc[:],
                                    None, op0=Alu.add)
            nextN = sb.tile([128, BL], fp32)
            nc.vector.tensor_scalar(nextN[:], pemb[:, 2 * BL:3 * BL], bn_cc[:],
                                    None, op0=Alu.add)
            depotT = sb.tile([128, BL], fp32)
            nc.vector.tensor_scalar(depotT[:], pemb[:, 3 * BL:4 * BL],
                                    bd_cc[:], None, op0=Alu.add)

            # broadcast is_depot rows across partitions: [128, 64]
            iszbc_p = ps.tile([128, 2 * BL], fp32, tag="pt")
            nc.tensor.matmul(iszbc_p[:, 0:BL], lhsT=ones1[:],
                             rhs=iszT[0:1, 0:BL], start=True, stop=True)
            nc.tensor.matmul(iszbc_p[:, BL:2 * BL], lhsT=ones1[:],
                             rhs=iszT[0:1, BL:2 * BL], start=True, stop=True)

            # blend curr/next: emb = node + isz*(depot - node)
            d1 = sb.tile([128, BL], fp32)
            nc.vector.tensor_tensor(d1[:], depotT[:], currN[:], op=Alu.subtract)
            d2 = sb.tile([128, BL], fp32)
            nc.vector.tensor_tensor(d2[:], d1[:], iszbc_p[:, 0:BL], op=Alu.mult)
            currT = sb.tile([128, BL], fp32)
            nc.vector.tensor_tensor(currT[:], currN[:], d2[:], op=Alu.add)
            d3 = sb.tile([128, BL], fp32)
            nc.vector.tensor_tensor(d3[:], depotT[:], nextN[:], op=Alu.subtract)
            d4 = sb.tile([128, BL], fp32)
            nc.vector.tensor_tensor(d4[:], d3[:], iszbc_p[:, BL:2 * BL],
                                    op=Alu.mult)
            nextT = sb.tile([128, BL], fp32)
            nc.vector.tensor_tensor(nextT[:], nextN[:], d4[:], op=Alu.add)

            # ---------------- q, g, w37 ----------------
            qT_p = psq.tile([128, BL], fp32, tag="pqg")
            nc.tensor.matmul(qT_p[:], lhsT=wq_c[:, 0:EMB], rhs=currT[:],
                             start=True, stop=False)
            nc.tensor.matmul(qT_p[:], lhsT=wq_c[:, EMB:2 * EMB], rhs=nextT[:],
                             start=False, stop=False)
            nc.tensor.matmul(qT_p[:], lhsT=wq_c[:, 2 * EMB:3 * EMB],
                             rhs=stateT[:], start=False, stop=True)
            qT = sb.tile([128, BL], fp32)
            nc.vector.tensor_copy(qT[:], qT_p[:])

            gT_p = psq.tile([128, BL], fp32, tag="pqg")
            nc.tensor.matmul(gT_p[:], lhsT=wk2T[:], rhs=qT[:],
                             start=True, stop=True)
            gT = sb.tile([128, BL], fp32)
            nc.vector.tensor_copy(gT[:], gT_p[:])

            # repeat g columns 4x: col p = j*BL + b  ->  g[:, b]
            gT4 = sb.tile([128, 128], fp32)
            nc.vector.tensor_copy(
                gT4[:].rearrange("p (j b) -> p j b", j=J),
                gT[:].unsqueeze(1).broadcast_to([128, J, BL]))

            # w37 [128,7]: 0:3 w3 | 3 cn | 4:6 wd | 6 cd   (per partition's b)
            w37_p = psq.tile([128, 7], fp32, tag="pqg")
            nc.tensor.matmul(w37_p[:], lhsT=gT4[:], rhs=w7T[:],
                             start=True, stop=True)
            w37 = sb.tile([128, 7], fp32)
            nc.vector.tensor_copy(w37[:], w37_p[:])
            biasN = sb.tile([128, 1], fp32)
            nc.vector.tensor_scalar(biasN[:], w37[:, 3:4], NORM, None,
                                    op0=Alu.mult)

            # ---------------- logits stream ----------------
            maskb = sb.tile([128, NF], fp32)
            for k in range(KC):
                ksl = slice(k * NFC, (k + 1) * NFC)
                nc.vector.tensor_scalar(maskb[:, ksl], mku[:, ksl], MASK_BIG,
                                        MASK_BIG, op0=Alu.mult,
                                        op1=Alu.subtract)
            tmp = sb.tile([128, NF * 3], fp32)
            for k in range(KC):
                ksl = slice(k * NFC * 3, (k + 1) * NFC * 3)
                nc.vector.tensor_tensor(
                    tmp[:, ksl].rearrange("p (f c) -> p f c", c=3),
                    x[:, ksl].rearrange("p (f c) -> p f c", c=3),
                    w37[:, 0:3].unsqueeze(1).broadcast_to([128, NFC, 3]),
                    op=Alu.mult)
            L = sb.tile([128, NF], fp32)
            nc.vector.tensor_reduce(
                L[:], tmp[:].rearrange("p (f c) -> p f c", c=3),
                axis=X, op=Alu.add)

            # depot logit at n=0 (rows 0:BL are the j=0 block, b-ordered):
            # L[b,0] = xd.wd + cd - cn  (cn re-added by the exp bias)
            cdmn = sb.tile([BL, 1], fp32)
            nc.vector.tensor_tensor(cdmn[:], w37[0:BL, 6:7], w37[0:BL, 3:4],
                                    op=Alu.subtract)
            dlm = sb.tile([BL, 2], fp32)
            nc.vector.tensor_tensor(dlm[:], x03[:, 0:2], w37[0:BL, 4:6],
                                    op=Alu.mult)
            dlr = sb.tile([BL, 1], fp32)
            nc.vector.tensor_reduce(dlr[:], dlm[:], axis=X, op=Alu.add)
            nc.vector.tensor_tensor(L[0:BL, 0:1], dlr[:], cdmn[:], op=Alu.add)

            nc.vector.tensor_tensor(L[:], L[:], maskb[:], op=Alu.add)

            # E = exp(NORM*L + NORM*cn); accum_out -> row sums
            E = sb.tile([128, NF], fp32)
            sjact = sb.tile([128, 1], fp32)
            nc.scalar.activation(E[:], L[:], Act.Exp, bias=biasN[:],
                                 scale=NORM, accum_out=sjact[:])
            s3S = sb.tile([128, 4], fp32)
            nc.vector.tensor_copy(s3S[:, 3:4], sjact[:])

            # s3 partials: s3S[:,0:3] = sum_f E * x
            tmp2 = sb.tile([128, NF * 3], fp32)
            nc.vector.tensor_tensor(
                tmp2[:].rearrange("p (f c) -> p f c", c=3),
                x[:].rearrange("p (f c) -> p f c", c=3),
                E[:].unsqueeze(2).broadcast_to([128, NF, 3]), op=Alu.mult)
            nc.vector.tensor_reduce(
                s3S[:, 0:3], tmp2[:].rearrange("p (f c) -> p c f", c=3),
                axis=X, op=Alu.add)

            # cross-j reduction: S[b,:] = sum_j s3S[j*BL+b,:]
            s3b_p = ps.tile([BL, 4], fp32, tag="pt")
            nc.tensor.matmul(s3b_p[:], lhsT=rep_eye[:], rhs=s3S[:],
                             start=True, stop=True)

            recipS = sb.tile([BL, 1], fp32)
            nc.vector.reciprocal(recipS[:], s3b_p[:, 3:4])

            # urhs [32,7]: 0:3 s3 | 3 1-a0 | 4:6 a0*xd | 6 a0
            urhs = sb.tile([BL, 7], fp32)
            nc.vector.tensor_scalar(urhs[:, 6:7], E[0:BL, 0:1], recipS[:],
                                    None, op0=Alu.mult)
            t1 = sb.tile([BL, 3], fp32)
            nc.vector.tensor_scalar(t1[:], s3b_p[:, 0:3], recipS[:], None,
                                    op0=Alu.mult)
            t2 = sb.tile([BL, 3], fp32)
            nc.vector.tensor_scalar(t2[:], x0_c[:], urhs[:, 6:7], None,
                                    op0=Alu.mult)
            nc.vector.tensor_tensor(urhs[:, 0:3], t1[:], t2[:],
                                    op=Alu.subtract)
            nc.vector.tensor_scalar(urhs[:, 3:4], urhs[:, 6:7], -1.0, 1.0,
                                    op0=Alu.mult, op1=Alu.add)
            nc.vector.tensor_copy(urhs[:, 4:6], t2[:, 0:2])

            urT_p = ps.tile([7, BL], fp32, tag="pt")
            nc.tensor.transpose(urT_p[:], urhs[:], idb)
            urT = sb.tile([7, BL], fp32)
            nc.vector.tensor_copy(urT[:], urT_p[:])

            u_p = psq.tile([128, BL], fp32, tag="pqg")
            nc.tensor.matmul(u_p[:], lhsT=w7c[:], rhs=urT[:],
                             start=True, stop=True)
            u_sb = sb.tile([128, BL], fp32)
            nc.vector.tensor_copy(u_sb[:], u_p[:])

            hT_p = psq.tile([128, BL], fp32, tag="pqg")
            nc.tensor.matmul(hT_p[:], lhsT=wv2_c[:], rhs=u_sb[:],
                             start=True, stop=False)
            nc.tensor.matmul(hT_p[:], lhsT=wv1_c[:], rhs=currT[:],
                             start=False, stop=True)
            hT = sb.tile([128, BL], fp32)
            nc.vector.tensor_copy(hT[:], hT_p[:])
            h_p = ps.tile([BL, 128], fp32, tag="pt")
            nc.tensor.transpose(h_p[:], hT[:], ident[:])
            h_sb = sb.tile([BL, 128], fp32)
            nc.vector.tensor_copy(h_sb[:], h_p[:])
            nc.sync.dma_start(out[:], h_sb[:])

    if finalize:
        nc.finalize()
    return nc


def _shard_inputs(node_feats, state, W_node, b_node, W_depot, b_depot,
                  W_state, b_state, w_q, w_k, w_v, curr_node_id,
                  next_node_id, mask):
    f32 = np.float32
    node_feats = np.ascontiguousarray(node_feats, dtype=f32)
    state = np.ascontiguousarray(state, dtype=f32)
    mask_u8 = np.ascontiguousarray(mask).astype(np.uint8)
    ids = np.stack([np.asarray(curr_node_id), np.asarray(next_node_id)],
                   axis=1).astype(np.int32)
    wk2 = np.ascontiguousarray(w_k[EMB:2 * EMB], dtype=f32)
    w7s = np.concatenate([
        np.asarray(W_node, dtype=f32),
        np.asarray(b_node, dtype=f32)[None, :],
        np.asarray(W_depot, dtype=f32),
        np.asarray(b_depot, dtype=f32)[None, :],
    ], axis=0)
    shared = {
        "W_node": np.ascontiguousarray(W_node, dtype=f32),
        "b_node": np.ascontiguousarray(b_node, dtype=f32),
        "W_depot": np.ascontiguousarray(W_depot, dtype=f32),
        "b_depot": np.ascontiguousarray(b_depot, dtype=f32),
        "w7s": np.ascontiguousarray(w7s),
        "W_state": np.ascontiguousarray(W_state, dtype=f32),
        "b_state": np.ascontiguousarray(b_state, dtype=f32),
        "w_q": np.ascontiguousarray(w_q, dtype=f32),
        "wk2": wk2,
        "w_v": np.ascontiguousarray(w_v, dtype=f32),
    }
    in_maps = []
    for i in range(NCORES):
        s = slice(i * BL, (i + 1) * BL)
        m = dict(shared)
        m["node_feats"] = np.ascontiguousarray(node_feats[s])
        m["state"] = np.ascontiguousarray(state[s])
        m["ids"] = np.ascontiguousarray(ids[s])
        m["mask_u8"] = np.ascontiguousarray(mask_u8[s])
        in_maps.append(m)
    return in_maps


def _run(inputs, trace=False):
    from concourse.bass_utils import run_bass_kernel_spmd

    if "nc" not in _CACHE:
        _CACHE["nc"] = _build()
    nc = _CACHE["nc"]
    in_maps = _shard_inputs(**inputs)
    res = run_bass_kernel_spmd(nc, in_maps, core_ids=list(range(NCORES)),
                               trace=trace)
    full = np.concatenate([r["out"] for r in res.results], axis=0)
    return full, res


def kernel(**inputs):
    full, _ = _run(inputs, trace=False)
    return full


# revision 27
# speedup vs baseline: 1.2022x; 1.2022x over previous
"""Trainium2 Bass kernel for nn_AttentionGraphEncoder (gnn_message_passing).

Math restructure (exact, not approximate):
  The reference computes, per batch b:
    nf[n]   = embed(node_feats[b,n])                  # [N, E] affine in x
    k[n]    = curr_emb @ Wk1 + nf[n] @ Wk2
    logits  = NORM * q . k[n]  -> softmax (masked)
    v[n]    = curr_emb @ Wv1 + nf[n] @ Wv2
    h       = sum_n attn[n] v[n]
  Since q.k[n] = const + nf[n].(Wk2 @ q) and nf[n] is affine in the raw
  3-dim node coordinates, the logits reduce to x[n] . w3(b) + const with
  w3(b) = W_node @ Wk2 @ q(b)  (a 3-vector per batch).  Likewise
    h = curr_emb @ Wv1 + (a0*depot_emb + (1-a0)*b_node + s3 @ W_node) @ Wv2
  with s3 = sum_{n>=1} attn[n] x[n] (a 3-vector) and a0 = attn[0].
  So the big [N,2E]@[2E,E] matmuls disappear; the kernel streams
  node_feats ([B,N,3], shipped bf16) once through the DVE and does tiny
  matmuls.

Sharding: pure data parallel, batch 256 -> 32 per core across 8 cores.

Perf notes:
  - DMA issue blocks the issuing engine's queue for the transfer, so the
    x stream is split across the Scalar and Tensor queues while Sync
    carries the (packed) weights; ids goes absolutely first (it heads
    the longest dependency chain: gather -> q -> w3 -> logits).
  - Small weights are packed host-side into three tensors so the Sync
    queue issues 5 DMAs instead of ~15.
  - The node stream is bf16 (halves HBM traffic, 2x DVE rate); all
    reductions accumulate in f32.
"""

import math

import numpy as np

B, N, NODE_DIM, STATE_DIM, EMB = 256, 2048, 3, 4, 128
NCORES = 8
BL = B // NCORES          # 32 batch elements per core
J = 4                     # node-chunks per batch -> 128 partitions (j*BL + b)
NF = N // J               # 512 nodes per partition row
KC = 4                    # free-dim DMA/compute chunks of the x stream
NFC = NF // KC            # 128 nodes per chunk
NORM = 1.0 / math.sqrt(EMB)
MASK_BIG = 400.0          # pre-NORM additive mask magnitude (400*NORM ~ 35)
WBIG_COLS = 3 * EMB + EMB + 2 * EMB + 3   # wq | wk2 | wv | bias cols = 771

_CACHE = {}


def _build(finalize=True):
    import concourse.bacc as bacc
    import concourse.bass as bass
    import concourse.mybir as mybir
    import concourse.tile as tile
    from concourse.masks import make_identity

    fp32 = mybir.dt.float32
    bf16 = mybir.dt.bfloat16
    i32 = mybir.dt.int32
    u8 = mybir.dt.uint8
    Alu = mybir.AluOpType
    Act = mybir.ActivationFunctionType
    X = mybir.AxisListType.X

    nc = bacc.Bacc("TRN2")

    nfd = nc.dram_tensor("node_feats", [BL, N, NODE_DIM], bf16,
                         kind="ExternalInput")
    # bpack: state(4) | x0(3)  per batch row, f32
    bpk = nc.dram_tensor("bpack", [BL, 7], fp32, kind="ExternalInput")
    # wpack [7, 512]: w7s(128) | W_state(128, rows 0:4) | W_node(128, rows
    # 0:3) | W_depot(128, rows 0:2)
    wpk = nc.dram_tensor("wpack", [7, 512], fp32, kind="ExternalInput")
    # wbig [128, 771]: wq(384) | wk2(128) | wv1(128) | wv2(128) | bias cols(3)
    wbg = nc.dram_tensor("wbig", [128, WBIG_COLS], fp32, kind="ExternalInput")
    ids = nc.dram_tensor("ids", [BL, 2], i32, kind="ExternalInput")
    mk = nc.dram_tensor("mask_u8", [BL, N], u8, kind="ExternalInput")
    out = nc.dram_tensor("out", [BL, EMB], fp32, kind="ExternalOutput")

    with tile.TileContext(nc, pool_alloc_mode="queue") as tc:
        with (
            tc.tile_pool(name="sb", bufs=1) as sb,
            tc.tile_pool(name="ps", bufs=3, space="PSUM") as ps,
            tc.tile_pool(name="pse", bufs=1, space="PSUM") as pse,
            tc.tile_pool(name="psq", bufs=2, space="PSUM") as psq,
        ):
            # ------- gpsimd constants (all before the PE warm-up) -------
            ident = sb.tile([128, 128], fp32)
            make_identity(nc, ident[:])
            ones1 = sb.tile([1, 128], fp32)
            nc.gpsimd.memset(ones1[:], 1.0)
            iota_p = sb.tile([BL, 1], i32)
            nc.gpsimd.iota(iota_p[:], pattern=[[0, 1]], base=0,
                           channel_multiplier=N)
            # rep_eye[p, y] = 1 iff p % BL == y  (cross-j reduce as a matmul)
            rep_eye = sb.tile([128, BL], fp32)
            nc.gpsimd.memset(rep_eye[:], 0.0)
            for j in range(J):
                nc.gpsimd.affine_select(
                    out=rep_eye[:], in_=rep_eye[:],
                    compare_op=Alu.not_equal, fill=1.0,
                    base=-BL * j, pattern=[[-1, BL]], channel_multiplier=1)
            # PE warm-up: one op depending on the LAST gpsimd constant so
            # later PE ops see all Pool ticks as observed.
            junk_p = ps.tile([1, 1], fp32, tag="pt")
            nc.tensor.matmul(junk_p[:], lhsT=rep_eye[:, 0:1],
                             rhs=ident[:, 0:1], start=True, stop=True)

            # ------------------- input DMAs -------------------
            # Sync queue, most-critical first.
            ids_sb = sb.tile([BL, 2], i32)
            nc.sync.dma_start(ids_sb[:], ids[:])
            bp_sb = sb.tile([BL, 7], fp32)
            nc.sync.dma_start(bp_sb[:], bpk[:])
            wp_sb = sb.tile([7, 512], fp32)
            nc.sync.dma_start(wp_sb[:], wpk[:])
            wb_sb = sb.tile([128, WBIG_COLS], fp32)
            nc.sync.dma_start(wb_sb[:], wbg[:])
            # node stream (bf16) split across the Scalar + Sync queues
            x = sb.tile([128, NF * 3], bf16)
            xview = (nfd[:].rearrange("b (j f) c -> b j (f c)", j=J)
                     .transpose([1, 0, 2]))                 # [J, BL, NF*3]
            for k in range(KC):
                ksl = slice(k * NFC * 3, (k + 1) * NFC * 3)
                eng = nc.scalar if k % 2 == 0 else nc.sync
                eng.dma_start(x[:, ksl], xview[:, :, ksl])
            mku = sb.tile([128, NF], u8)
            mview = (mk[:].rearrange("b (j f) -> b j f", j=J)
                     .transpose([1, 0, 2]))                 # [J, BL, NF]
            nc.scalar.dma_start(mku[:], mview[:])

            # ------------- DVE funnels (1 DMA dep each) -------------
            ids_c = sb.tile([BL, 2], i32)
            nc.vector.tensor_copy(ids_c[:], ids_sb[:])
            bp_c = sb.tile([BL, 7], fp32)
            nc.vector.tensor_copy(bp_c[:], bp_sb[:])
            wp_c = sb.tile([7, 512], fp32)
            nc.vector.tensor_copy(wp_c[:], wp_sb[:])
            wb_c = sb.tile([128, WBIG_COLS], fp32)
            nc.vector.tensor_copy(wb_c[:], wb_sb[:])
            w7c = wp_c[:, 0:128]
            ws_c = wp_c[0:4, 128:256]
            wn3_c = wp_c[0:3, 256:384]
            wd2_c = wp_c[0:2, 384:512]
            wq_c = wb_c[:, 0:3 * EMB]
            wk2_c = wb_c[:, 3 * EMB:4 * EMB]
            wv1_c = wb_c[:, 4 * EMB:5 * EMB]
            wv2_c = wb_c[:, 5 * EMB:6 * EMB]
            bn_cc = wb_c[:, 6 * EMB:6 * EMB + 1]
            bd_cc = wb_c[:, 6 * EMB + 1:6 * EMB + 2]
            bs_cc = wb_c[:, 6 * EMB + 2:6 * EMB + 3]

            # ---------------- gathers ----------------
            offs = sb.tile([BL, 2], i32)
            nc.vector.tensor_tensor(offs[:], ids_c[:],
                                    iota_p[:].to_broadcast([BL, 2]),
                                    op=Alu.add)
            nfr = nfd[:].rearrange("b n c -> (b n) c")
            xc3 = sb.tile([BL, NODE_DIM], bf16)
            nc.gpsimd.indirect_dma_start(
                out=xc3[:], out_offset=None, in_=nfr,
                in_offset=bass.IndirectOffsetOnAxis(ap=offs[:, 0:1], axis=0))
            xn3 = sb.tile([BL, NODE_DIM], bf16)
            nc.gpsimd.indirect_dma_start(
                out=xn3[:], out_offset=None, in_=nfr,
                in_offset=bass.IndirectOffsetOnAxis(ap=offs[:, 1:2], axis=0))
            xc_c = sb.tile([BL, NODE_DIM], fp32)
            nc.vector.tensor_copy(xc_c[:], xc3[:])
            xn_c = sb.tile([BL, NODE_DIM], fp32)
            nc.vector.tensor_copy(xn_c[:], xn3[:])
            isz2 = sb.tile([BL, 2], fp32)
            nc.vector.tensor_scalar(isz2[:], ids_c[:], 0, None,
                                    op0=Alu.is_equal)

            # ------------- transposes to [feat, b] -------------
            idb = ident[0:BL, 0:BL]
            stT_p = ps.tile([STATE_DIM, BL], fp32, tag="pt")
            nc.tensor.transpose(stT_p[:], bp_c[:, 0:4], idb)
            stT = sb.tile([STATE_DIM, BL], fp32)
            nc.vector.tensor_copy(stT[:], stT_p[:])
            xcT_p = ps.tile([NODE_DIM, BL], fp32, tag="pt")
            nc.tensor.transpose(xcT_p[:], xc_c[:], idb)
            xcT = sb.tile([NODE_DIM, BL], fp32)
            nc.vector.tensor_copy(xcT[:], xcT_p[:])
            xnT_p = ps.tile([NODE_DIM, BL], fp32, tag="pt")
            nc.tensor.transpose(xnT_p[:], xn_c[:], idb)
            xnT = sb.tile([NODE_DIM, BL], fp32)
            nc.vector.tensor_copy(xnT[:], xnT_p[:])
            x0T_p = ps.tile([NODE_DIM, BL], fp32, tag="pt")
            nc.tensor.transpose(x0T_p[:], bp_c[:, 4:7], idb)
            x0T = sb.tile([NODE_DIM, BL], fp32)
            nc.vector.tensor_copy(x0T[:], x0T_p[:])
            iszT_p = ps.tile([1, 2 * BL], fp32, tag="pt")
            nc.tensor.transpose(iszT_p[0:1, 0:BL], isz2[:, 0:1], idb)
            nc.tensor.transpose(iszT_p[0:1, BL:2 * BL], isz2[:, 1:2], idb)
            iszT = sb.tile([1, 2 * BL], fp32)
            nc.vector.tensor_copy(iszT[:], iszT_p[:])
            wk2T_p = ps.tile([128, 128], fp32, tag="pt")
            nc.tensor.transpose(wk2T_p[:], wk2_c, ident[:])
            wk2T = sb.tile([128, 128], fp32)
            nc.vector.tensor_copy(wk2T[:], wk2T_p[:])
            w7T_p = ps.tile([128, 7], fp32, tag="pt")
            nc.tensor.transpose(w7T_p[:], w7c, ident[0:7, 0:7])
            w7T = sb.tile([128, 7], fp32)
            nc.vector.tensor_copy(w7T[:], w7T_p[:])

            # ---------- embeddings (transposed [E, b]) ----------
            pemb = pse.tile([128, 4 * BL], fp32, tag="pemb")
            nc.tensor.matmul(pemb[:, 0:BL], lhsT=ws_c, rhs=stT[:],
                             start=True, stop=True)
            nc.tensor.matmul(pemb[:, BL:2 * BL], lhsT=wn3_c, rhs=xcT[:],
                             start=True, stop=True)
            nc.tensor.matmul(pemb[:, 2 * BL:3 * BL], lhsT=wn3_c, rhs=xnT[:],
                             start=True, stop=True)
            nc.tensor.matmul(pemb[:, 3 * BL:4 * BL], lhsT=wd2_c,
                             rhs=x0T[0:2, :], start=True, stop=True)
            stateT = sb.tile([128, BL], fp32)
            nc.vector.tensor_scalar(stateT[:], pemb[:, 0:BL], bs_cc, None,
                                    op0=Alu.add)
            currN = sb.tile([128, BL], fp32)
            nc.vector.tensor_scalar(currN[:], pemb[:, BL:2 * BL], bn_cc,
                                    None, op0=Alu.add)
            nextN = sb.tile([128, BL], fp32)
            nc.vector.tensor_scalar(nextN[:], pemb[:, 2 * BL:3 * BL], bn_cc,
                                    None, op0=Alu.add)
            depotT = sb.tile([128, BL], fp32)
            nc.vector.tensor_scalar(depotT[:], pemb[:, 3 * BL:4 * BL],
                                    bd_cc, None, op0=Alu.add)

            # broadcast is_depot rows across partitions: [128, 64]
            iszbc_p = ps.tile([128, 2 * BL], fp32, tag="pt")
            nc.tensor.matmul(iszbc_p[:, 0:BL], lhsT=ones1[:],
                             rhs=iszT[0:1, 0:BL], start=True, stop=True)
            nc.tensor.matmul(iszbc_p[:, BL:2 * BL], lhsT=ones1[:],
                             rhs=iszT[0:1, BL:2 * BL], start=True, stop=True)

            # blend curr/next: emb = node + isz*(depot - node)
            d1 = sb.tile([128, BL], fp32)
            nc.vector.tensor_tensor(d1[:], depotT[:], currN[:], op=Alu.subtract)
            d2 = sb.tile([128, BL], fp32)
            nc.vector.tensor_tensor(d2[:], d1[:], iszbc_p[:, 0:BL], op=Alu.mult)
            currT = sb.tile([128, BL], fp32)
            nc.vector.tensor_tensor(currT[:], currN[:], d2[:], op=Alu.add)
            d3 = sb.tile([128, BL], fp32)
            nc.vector.tensor_tensor(d3[:], depotT[:], nextN[:], op=Alu.subtract)
            d4 = sb.tile([128, BL], fp32)
            nc.vector.tensor_tensor(d4[:], d3[:], iszbc_p[:, BL:2 * BL],
                                    op=Alu.mult)
            nextT = sb.tile([128, BL], fp32)
            nc.vector.tensor_tensor(nextT[:], nextN[:], d4[:], op=Alu.add)

            # ---------------- q, g, w37 ----------------
            qT_p = psq.tile([128, BL], fp32, tag="pqg")
            nc.tensor.matmul(qT_p[:], lhsT=wq_c[:, 0:EMB], rhs=currT[:],
                             start=True, stop=False)
            nc.tensor.matmul(qT_p[:], lhsT=wq_c[:, EMB:2 * EMB], rhs=nextT[:],
                             start=False, stop=False)
            nc.tensor.matmul(qT_p[:], lhsT=wq_c[:, 2 * EMB:3 * EMB],
                             rhs=stateT[:], start=False, stop=True)
            qT = sb.tile([128, BL], fp32)
            nc.vector.tensor_copy(qT[:], qT_p[:])

            gT_p = psq.tile([128, BL], fp32, tag="pqg")
            nc.tensor.matmul(gT_p[:], lhsT=wk2T[:], rhs=qT[:],
                             start=True, stop=True)
            gT = sb.tile([128, BL], fp32)
            nc.vector.tensor_copy(gT[:], gT_p[:])

            # repeat g columns 4x: col p = j*BL + b  ->  g[:, b]
            gT4 = sb.tile([128, 128], fp32)
            nc.vector.tensor_copy(
                gT4[:].rearrange("p (j b) -> p j b", j=J),
                gT[:].unsqueeze(1).broadcast_to([128, J, BL]))

            # w37 [128,7]: 0:3 w3 | 3 cn | 4:6 wd | 6 cd   (per partition's b)
            w37_p = psq.tile([128, 7], fp32, tag="pqg")
            nc.tensor.matmul(w37_p[:], lhsT=gT4[:], rhs=w7T[:],
                             start=True, stop=True)
            w37 = sb.tile([128, 7], fp32)
            nc.vector.tensor_copy(w37[:], w37_p[:])
            biasN = sb.tile([128, 1], fp32)
            nc.vector.tensor_scalar(biasN[:], w37[:, 3:4], NORM, None,
                                    op0=Alu.mult)
            w33b = sb.tile([128, 3], bf16)
            nc.vector.tensor_copy(w33b[:], w37[:, 0:3])

            # ---------------- logits stream ----------------
            # mask bias on gpsimd (parallel with the DVE stream)
            maskb = sb.tile([128, NF], fp32)
            nc.gpsimd.tensor_scalar(maskb[:], mku[:], MASK_BIG, MASK_BIG,
                                    op0=Alu.mult, op1=Alu.subtract)
            tmp = sb.tile([128, NF * 3], bf16)
            for k in range(KC):
                ksl = slice(k * NFC * 3, (k + 1) * NFC * 3)
                nc.vector.tensor_tensor(
                    tmp[:, ksl].rearrange("p (f c) -> p f c", c=3),
                    x[:, ksl].rearrange("p (f c) -> p f c", c=3),
                    w33b[:].unsqueeze(1).broadcast_to([128, NFC, 3]),
                    op=Alu.mult)
            L = sb.tile([128, NF], fp32)
            nc.vector.tensor_reduce(
                L[:], tmp[:].rearrange("p (f c) -> p f c", c=3),
                axis=X, op=Alu.add)

            # depot logit at n=0 (rows 0:BL are the j=0 block, b-ordered):
            # L[b,0] = xd.wd + cd - cn  (cn re-added by the exp bias)
            cdmn = sb.tile([BL, 1], fp32)
            nc.vector.tensor_tensor(cdmn[:], w37[0:BL, 6:7], w37[0:BL, 3:4],
                                    op=Alu.subtract)
            dlm = sb.tile([BL, 2], fp32)
            nc.vector.tensor_tensor(dlm[:], bp_c[:, 4:6], w37[0:BL, 4:6],
                                    op=Alu.mult)
            dlr = sb.tile([BL, 1], fp32)
            nc.vector.tensor_reduce(dlr[:], dlm[:], axis=X, op=Alu.add)
            nc.vector.tensor_tensor(L[0:BL, 0:1], dlr[:], cdmn[:], op=Alu.add)

            nc.vector.tensor_tensor(L[:], L[:], maskb[:], op=Alu.add)

            # E = exp(NORM*L + NORM*cn); accum_out -> row sums (f32)
            E = sb.tile([128, NF], bf16)
            sjact = sb.tile([128, 1], fp32)
            nc.scalar.activation(E[:], L[:], Act.Exp, bias=biasN[:],
                                 scale=NORM, accum_out=sjact[:])
            s3S = sb.tile([128, 4], fp32)
            nc.vector.tensor_copy(s3S[:, 3:4], sjact[:])

            # s3 partials: s3S[:,0:3] = sum_f E * x
            tmp2 = sb.tile([128, NF * 3], bf16)
            nc.vector.tensor_tensor(
                tmp2[:].rearrange("p (f c) -> p f c", c=3),
                x[:].rearrange("p (f c) -> p f c", c=3),
                E[:].unsqueeze(2).broadcast_to([128, NF, 3]), op=Alu.mult)
            nc.vector.tensor_reduce(
                s3S[:, 0:3], tmp2[:].rearrange("p (f c) -> p c f", c=3),
                axis=X, op=Alu.add)

            # cross-j reduction: S[b,:] = sum_j s3S[j*BL+b,:]
            s3b_p = ps.tile([BL, 4], fp32, tag="pt")
            nc.tensor.matmul(s3b_p[:], lhsT=rep_eye[:], rhs=s3S[:],
                             start=True, stop=True)

            recipS = sb.tile([BL, 1], fp32)
            nc.vector.reciprocal(recipS[:], s3b_p[:, 3:4])

            # urhs [32,7]: 0:3 s3 | 3 1-a0 | 4:6 a0*xd | 6 a0
            urhs = sb.tile([BL, 7], fp32)
            nc.vector.tensor_scalar(urhs[:, 6:7], E[0:BL, 0:1], recipS[:],
                                    None, op0=Alu.mult)
            t1 = sb.tile([BL, 3], fp32)
            nc.vector.tensor_scalar(t1[:], s3b_p[:, 0:3], recipS[:], None,
                                    op0=Alu.mult)
            t2 = sb.tile([BL, 3], fp32)
            nc.vector.tensor_scalar(t2[:], bp_c[:, 4:7], urhs[:, 6:7], None,
                                    op0=Alu.mult)
            nc.vector.tensor_tensor(urhs[:, 0:3], t1[:], t2[:],
                                    op=Alu.subtract)
            nc.vector.tensor_scalar(urhs[:, 3:4], urhs[:, 6:7], -1.0, 1.0,
                                    op0=Alu.mult, op1=Alu.add)
            nc.vector.tensor_copy(urhs[:, 4:6], t2[:, 0:2])

            urT_p = ps.tile([7, BL], fp32, tag="pt")
            nc.tensor.transpose(urT_p[:], urhs[:], idb)
            urT = sb.tile([7, BL], fp32)
            nc.vector.tensor_copy(urT[:], urT_p[:])

            u_p = psq.tile([128, BL], fp32, tag="pqg")
            nc.tensor.matmul(u_p[:], lhsT=w7c, rhs=urT[:],
                             start=True, stop=True)
            u_sb = sb.tile([128, BL], fp32)
            nc.vector.tensor_copy(u_sb[:], u_p[:])

            hT_p = psq.tile([128, BL], fp32, tag="pqg")
            nc.tensor.matmul(hT_p[:], lhsT=wv2_c, rhs=u_sb[:],
                             start=True, stop=False)
            nc.tensor.matmul(hT_p[:], lhsT=wv1_c, rhs=currT[:],
                             start=False, stop=True)
            hT = sb.tile([128, BL], fp32)
            nc.vector.tensor_copy(hT[:], hT_p[:])
            h_p = ps.tile([BL, 128], fp32, tag="pt")
            nc.tensor.transpose(h_p[:], hT[:], ident[:])
            h_sb = sb.tile([BL, 128], fp32)
            nc.vector.tensor_copy(h_sb[:], h_p[:])
            nc.sync.dma_start(out[:], h_sb[:])

    if finalize:
        nc.finalize()
    return nc


def _shard_inputs(node_feats, state, W_node, b_node, W_depot, b_depot,
                  W_state, b_state, w_q, w_k, w_v, curr_node_id,
                  next_node_id, mask):
    import ml_dtypes

    f32 = np.float32
    node_feats = np.ascontiguousarray(node_feats, dtype=f32)
    nf_bf16 = node_feats.astype(ml_dtypes.bfloat16)
    state = np.ascontiguousarray(state, dtype=f32)
    mask_u8 = np.ascontiguousarray(mask).astype(np.uint8)
    ids = np.stack([np.asarray(curr_node_id), np.asarray(next_node_id)],
                   axis=1).astype(np.int32)

    W_node = np.asarray(W_node, dtype=f32)
    b_node = np.asarray(b_node, dtype=f32)
    W_depot = np.asarray(W_depot, dtype=f32)
    b_depot = np.asarray(b_depot, dtype=f32)
    W_state = np.asarray(W_state, dtype=f32)
    b_state = np.asarray(b_state, dtype=f32)
    w_q = np.asarray(w_q, dtype=f32)
    w_k = np.asarray(w_k, dtype=f32)
    w_v = np.asarray(w_v, dtype=f32)

    w7s = np.concatenate([W_node, b_node[None, :], W_depot,
                          b_depot[None, :]], axis=0)           # [7,128]
    wpack = np.zeros((7, 512), f32)
    wpack[:, 0:128] = w7s
    wpack[0:4, 128:256] = W_state
    wpack[0:3, 256:384] = W_node
    wpack[0:2, 384:512] = W_depot
    wbig = np.zeros((128, WBIG_COLS), f32)
    wbig[:, 0:128] = w_q[0:128]
    wbig[:, 128:256] = w_q[128:256]
    wbig[:, 256:384] = w_q[256:384]
    wbig[:, 384:512] = w_k[128:256]
    wbig[:, 512:640] = w_v[0:128]
    wbig[:, 640:768] = w_v[128:256]
    wbig[:, 768] = b_node
    wbig[:, 769] = b_depot
    wbig[:, 770] = b_state

    shared = {
        "wpack": np.ascontiguousarray(wpack),
        "wbig": np.ascontiguousarray(wbig),
    }
    in_maps = []
    for i in range(NCORES):
        s = slice(i * BL, (i + 1) * BL)
        m = dict(shared)
        m["node_feats"] = np.ascontiguousarray(nf_bf16[s])
        bp = np.concatenate([state[s], node_feats[s, 0, :]], axis=1)
        m["bpack"] = np.ascontiguousarray(bp)
        m["ids"] = np.ascontiguousarray(ids[s])
        m["mask_u8"] = np.ascontiguousarray(mask_u8[s])
        in_maps.append(m)
    return in_maps


def _run(inputs, trace=False):
    from concourse.bass_utils import run_bass_kernel_spmd

    if "nc" not in _CACHE:
        _CACHE["nc"] = _build()
    nc = _CACHE["nc"]
    in_maps = _shard_inputs(**inputs)
    res = run_bass_kernel_spmd(nc, in_maps, core_ids=list(range(NCORES)),
                               trace=trace)
    full = np.concatenate([r["out"] for r in res.results], axis=0)
    return full, res


def kernel(**inputs):
    full, _ = _run(inputs, trace=False)
    return full


# revision 28
# speedup vs baseline: 1.3907x; 1.1568x over previous
"""Trainium2 Bass kernel for nn_AttentionGraphEncoder (gnn_message_passing).

Math restructure (exact, not approximate):
  The reference computes, per batch b:
    nf[n]   = embed(node_feats[b,n])                  # [N, E] affine in x
    k[n]    = curr_emb @ Wk1 + nf[n] @ Wk2
    logits  = NORM * q . k[n]  -> softmax (masked)
    v[n]    = curr_emb @ Wv1 + nf[n] @ Wv2
    h       = sum_n attn[n] v[n]
  Since q.k[n] = const + nf[n].(Wk2 @ q) and nf[n] is affine in the raw
  3-dim node coordinates, the logits reduce to x[n] . w3(b) + const with
  w3(b) = W_node @ Wk2 @ q(b)  (a 3-vector per batch).  Likewise
    h = curr_emb @ Wv1 + (a0*depot_emb + (1-a0)*b_node + s3 @ W_node) @ Wv2
  with s3 = sum_{n>=1} attn[n] x[n] (a 3-vector) and a0 = attn[0].
  So the big [N,2E]@[2E,E] matmuls disappear; the kernel streams
  node_feats ([B,N,3], shipped bf16) once through the DVE and does tiny
  matmuls.

Sharding: pure data parallel, batch 256 -> 32 per core across 8 cores.

Perf notes:
  - DMA issue blocks the issuing engine's queue for the transfer, so the
    x stream is split across the Scalar and Tensor queues while Sync
    carries the (packed) weights; ids goes absolutely first (it heads
    the longest dependency chain: gather -> q -> w3 -> logits).
  - Small weights are packed host-side into three tensors so the Sync
    queue issues 5 DMAs instead of ~15.
  - The node stream is bf16 (halves HBM traffic, 2x DVE rate); all
    reductions accumulate in f32.
"""

import math

import numpy as np

B, N, NODE_DIM, STATE_DIM, EMB = 256, 2048, 3, 4, 128
NCORES = 8
BL = B // NCORES          # 32 batch elements per core
J = 4                     # node-chunks per batch -> 128 partitions (j*BL + b)
NF = N // J               # 512 nodes per partition row
KC = 4                    # free-dim DMA/compute chunks of the x stream
NFC = NF // KC            # 128 nodes per chunk
NORM = 1.0 / math.sqrt(EMB)
MASK_BIG = 400.0          # pre-NORM additive mask magnitude (400*NORM ~ 35)
WBIG_COLS = 3 * EMB + EMB + 2 * EMB + 3   # wq | wk2 | wv | bias cols = 771

_CACHE = {}


def _build(finalize=True):
    import concourse.bacc as bacc
    import concourse.bass as bass
    import concourse.mybir as mybir
    import concourse.tile as tile
    from concourse.masks import make_identity

    fp32 = mybir.dt.float32
    bf16 = mybir.dt.bfloat16
    i32 = mybir.dt.int32
    u8 = mybir.dt.uint8
    Alu = mybir.AluOpType
    Act = mybir.ActivationFunctionType
    X = mybir.AxisListType.X

    nc = bacc.Bacc("TRN2")

    nfd = nc.dram_tensor("node_feats", [BL, N, NODE_DIM], bf16,
                         kind="ExternalInput")
    # bpack: state(4) | x0(3)  per batch row, f32
    bpk = nc.dram_tensor("bpack", [BL, 7], fp32, kind="ExternalInput")
    # wpack [7, 512]: w7s(128) | W_state(128, rows 0:4) | W_node(128, rows
    # 0:3) | W_depot(128, rows 0:2)
    wpk = nc.dram_tensor("wpack", [7, 512], fp32, kind="ExternalInput")
    # wbig [128, 771]: wq(384) | wk2(128) | wv1(128) | wv2(128) | bias cols(3)
    wbg = nc.dram_tensor("wbig", [128, WBIG_COLS], fp32, kind="ExternalInput")
    ids = nc.dram_tensor("ids", [BL, 2], i32, kind="ExternalInput")
    mk = nc.dram_tensor("mask_u8", [BL, N], u8, kind="ExternalInput")
    out = nc.dram_tensor("out", [BL, EMB], fp32, kind="ExternalOutput")

    with tile.TileContext(nc, pool_alloc_mode="queue") as tc:
        with (
            tc.tile_pool(name="sb", bufs=1) as sb,
            tc.tile_pool(name="ps", bufs=3, space="PSUM") as ps,
            tc.tile_pool(name="pse", bufs=1, space="PSUM") as pse,
            tc.tile_pool(name="psq", bufs=2, space="PSUM") as psq,
        ):
            # ------- gpsimd constants (all before the PE warm-up) -------
            ident = sb.tile([128, 128], fp32)
            make_identity(nc, ident[:])
            ones1 = sb.tile([1, 128], fp32)
            nc.gpsimd.memset(ones1[:], 1.0)
            iota_p = sb.tile([BL, 1], i32)
            nc.gpsimd.iota(iota_p[:], pattern=[[0, 1]], base=0,
                           channel_multiplier=N)
            # rep_eye[p, y] = 1 iff p % BL == y  (cross-j reduce as a matmul)
            rep_eye = sb.tile([128, BL], fp32)
            nc.gpsimd.memset(rep_eye[:], 0.0)
            for j in range(J):
                nc.gpsimd.affine_select(
                    out=rep_eye[:], in_=rep_eye[:],
                    compare_op=Alu.not_equal, fill=1.0,
                    base=-BL * j, pattern=[[-1, BL]], channel_multiplier=1)
            # PE warm-up: one op depending on the LAST gpsimd constant so
            # later PE ops see all Pool ticks as observed.
            junk_p = ps.tile([1, 1], fp32, tag="pt")
            nc.tensor.matmul(junk_p[:], lhsT=rep_eye[:, 0:1],
                             rhs=ident[:, 0:1], start=True, stop=True)

            # ------------------- input DMAs -------------------
            # Sync queue, most-critical first.
            ids_sb = sb.tile([BL, 2], i32)
            nc.sync.dma_start(ids_sb[:], ids[:])
            bp_sb = sb.tile([BL, 7], fp32)
            nc.sync.dma_start(bp_sb[:], bpk[:])
            wp_sb = sb.tile([7, 512], fp32)
            nc.sync.dma_start(wp_sb[:], wpk[:])
            wb_sb = sb.tile([128, WBIG_COLS], fp32)
            nc.sync.dma_start(wb_sb[:], wbg[:])
            # node stream (bf16) split across the Scalar + Sync queues
            x = sb.tile([128, NF * 3], bf16)
            xview = (nfd[:].rearrange("b (j f) c -> b j (f c)", j=J)
                     .transpose([1, 0, 2]))                 # [J, BL, NF*3]
            for k in range(KC):
                ksl = slice(k * NFC * 3, (k + 1) * NFC * 3)
                eng = nc.scalar if k % 2 == 0 else nc.sync
                eng.dma_start(x[:, ksl], xview[:, :, ksl])
            mku = sb.tile([128, NF], u8)
            mview = (mk[:].rearrange("b (j f) -> b j f", j=J)
                     .transpose([1, 0, 2]))                 # [J, BL, NF]
            nc.scalar.dma_start(mku[:], mview[:])

            # Bacc splits excess waits onto event semaphores, so engines can
            # consume the DMA'd tiles directly (no funnel copies needed).
            ids_c = ids_sb
            bp_c = bp_sb
            w7c = wp_sb[:, 0:128]
            ws_c = wp_sb[0:4, 128:256]
            wn3_c = wp_sb[0:3, 256:384]
            wd2_c = wp_sb[0:2, 384:512]
            wq_c = wb_sb[:, 0:3 * EMB]
            wk2_c = wb_sb[:, 3 * EMB:4 * EMB]
            wv1_c = wb_sb[:, 4 * EMB:5 * EMB]
            wv2_c = wb_sb[:, 5 * EMB:6 * EMB]
            bn_cc = wb_sb[:, 6 * EMB:6 * EMB + 1]
            bd_cc = wb_sb[:, 6 * EMB + 1:6 * EMB + 2]
            bs_cc = wb_sb[:, 6 * EMB + 2:6 * EMB + 3]

            # ---------------- gathers ----------------
            offs = sb.tile([BL, 2], i32)
            nc.vector.tensor_tensor(offs[:], ids_c[:],
                                    iota_p[:].to_broadcast([BL, 2]),
                                    op=Alu.add)
            nfr = nfd[:].rearrange("b n c -> (b n) c")
            xc3 = sb.tile([BL, NODE_DIM], bf16)
            nc.gpsimd.indirect_dma_start(
                out=xc3[:], out_offset=None, in_=nfr,
                in_offset=bass.IndirectOffsetOnAxis(ap=offs[:, 0:1], axis=0))
            xn3 = sb.tile([BL, NODE_DIM], bf16)
            nc.gpsimd.indirect_dma_start(
                out=xn3[:], out_offset=None, in_=nfr,
                in_offset=bass.IndirectOffsetOnAxis(ap=offs[:, 1:2], axis=0))
            xc_c = sb.tile([BL, NODE_DIM], fp32)
            nc.vector.tensor_copy(xc_c[:], xc3[:])
            xn_c = sb.tile([BL, NODE_DIM], fp32)
            nc.vector.tensor_copy(xn_c[:], xn3[:])
            isz2 = sb.tile([BL, 2], fp32)
            nc.vector.tensor_scalar(isz2[:], ids_c[:], 0, None,
                                    op0=Alu.is_equal)

            # ------------- transposes to [feat, b] -------------
            idb = ident[0:BL, 0:BL]
            stT_p = ps.tile([STATE_DIM, BL], fp32, tag="pt")
            nc.tensor.transpose(stT_p[:], bp_c[:, 0:4], idb)
            stT = sb.tile([STATE_DIM, BL], fp32)
            nc.vector.tensor_copy(stT[:], stT_p[:])
            xcT_p = ps.tile([NODE_DIM, BL], fp32, tag="pt")
            nc.tensor.transpose(xcT_p[:], xc_c[:], idb)
            xcT = sb.tile([NODE_DIM, BL], fp32)
            nc.vector.tensor_copy(xcT[:], xcT_p[:])
            xnT_p = ps.tile([NODE_DIM, BL], fp32, tag="pt")
            nc.tensor.transpose(xnT_p[:], xn_c[:], idb)
            xnT = sb.tile([NODE_DIM, BL], fp32)
            nc.vector.tensor_copy(xnT[:], xnT_p[:])
            x0T_p = ps.tile([NODE_DIM, BL], fp32, tag="pt")
            nc.tensor.transpose(x0T_p[:], bp_c[:, 4:7], idb)
            x0T = sb.tile([NODE_DIM, BL], fp32)
            nc.vector.tensor_copy(x0T[:], x0T_p[:])
            iszT_p = ps.tile([1, 2 * BL], fp32, tag="pt")
            nc.tensor.transpose(iszT_p[0:1, 0:BL], isz2[:, 0:1], idb)
            nc.tensor.transpose(iszT_p[0:1, BL:2 * BL], isz2[:, 1:2], idb)
            iszT = sb.tile([1, 2 * BL], fp32)
            nc.vector.tensor_copy(iszT[:], iszT_p[:])
            wk2T_p = ps.tile([128, 128], fp32, tag="pt")
            nc.tensor.transpose(wk2T_p[:], wk2_c, ident[:])
            wk2T = sb.tile([128, 128], fp32)
            nc.vector.tensor_copy(wk2T[:], wk2T_p[:])
            w7T_p = ps.tile([128, 7], fp32, tag="pt")
            nc.tensor.transpose(w7T_p[:], w7c, ident[0:7, 0:7])
            w7T = sb.tile([128, 7], fp32)
            nc.vector.tensor_copy(w7T[:], w7T_p[:])

            # ---------- embeddings (transposed [E, b]) ----------
            pemb = pse.tile([128, 4 * BL], fp32, tag="pemb")
            nc.tensor.matmul(pemb[:, 0:BL], lhsT=ws_c, rhs=stT[:],
                             start=True, stop=True)
            nc.tensor.matmul(pemb[:, BL:2 * BL], lhsT=wn3_c, rhs=xcT[:],
                             start=True, stop=True)
            nc.tensor.matmul(pemb[:, 2 * BL:3 * BL], lhsT=wn3_c, rhs=xnT[:],
                             start=True, stop=True)
            nc.tensor.matmul(pemb[:, 3 * BL:4 * BL], lhsT=wd2_c,
                             rhs=x0T[0:2, :], start=True, stop=True)
            stateT = sb.tile([128, BL], fp32)
            nc.vector.tensor_scalar(stateT[:], pemb[:, 0:BL], bs_cc, None,
                                    op0=Alu.add)
            currN = sb.tile([128, BL], fp32)
            nc.vector.tensor_scalar(currN[:], pemb[:, BL:2 * BL], bn_cc,
                                    None, op0=Alu.add)
            nextN = sb.tile([128, BL], fp32)
            nc.vector.tensor_scalar(nextN[:], pemb[:, 2 * BL:3 * BL], bn_cc,
                                    None, op0=Alu.add)
            depotT = sb.tile([128, BL], fp32)
            nc.vector.tensor_scalar(depotT[:], pemb[:, 3 * BL:4 * BL],
                                    bd_cc, None, op0=Alu.add)

            # broadcast is_depot rows across partitions: [128, 64]
            iszbc_p = ps.tile([128, 2 * BL], fp32, tag="pt")
            nc.tensor.matmul(iszbc_p[:, 0:BL], lhsT=ones1[:],
                             rhs=iszT[0:1, 0:BL], start=True, stop=True)
            nc.tensor.matmul(iszbc_p[:, BL:2 * BL], lhsT=ones1[:],
                             rhs=iszT[0:1, BL:2 * BL], start=True, stop=True)

            # blend curr/next: emb = node + isz*(depot - node)
            d1 = sb.tile([128, BL], fp32)
            nc.vector.tensor_tensor(d1[:], depotT[:], currN[:], op=Alu.subtract)
            d2 = sb.tile([128, BL], fp32)
            nc.vector.tensor_tensor(d2[:], d1[:], iszbc_p[:, 0:BL], op=Alu.mult)
            currT = sb.tile([128, BL], fp32)
            nc.vector.tensor_tensor(currT[:], currN[:], d2[:], op=Alu.add)
            d3 = sb.tile([128, BL], fp32)
            nc.vector.tensor_tensor(d3[:], depotT[:], nextN[:], op=Alu.subtract)
            d4 = sb.tile([128, BL], fp32)
            nc.vector.tensor_tensor(d4[:], d3[:], iszbc_p[:, BL:2 * BL],
                                    op=Alu.mult)
            nextT = sb.tile([128, BL], fp32)
            nc.vector.tensor_tensor(nextT[:], nextN[:], d4[:], op=Alu.add)

            # ---------------- q, g, w37 ----------------
            qT_p = psq.tile([128, BL], fp32, tag="pqg")
            nc.tensor.matmul(qT_p[:], lhsT=wq_c[:, 0:EMB], rhs=currT[:],
                             start=True, stop=False)
            nc.tensor.matmul(qT_p[:], lhsT=wq_c[:, EMB:2 * EMB], rhs=nextT[:],
                             start=False, stop=False)
            nc.tensor.matmul(qT_p[:], lhsT=wq_c[:, 2 * EMB:3 * EMB],
                             rhs=stateT[:], start=False, stop=True)
            qT = sb.tile([128, BL], fp32)
            nc.vector.tensor_copy(qT[:], qT_p[:])

            gT_p = psq.tile([128, BL], fp32, tag="pqg")
            nc.tensor.matmul(gT_p[:], lhsT=wk2T[:], rhs=qT[:],
                             start=True, stop=True)
            gT = sb.tile([128, BL], fp32)
            nc.vector.tensor_copy(gT[:], gT_p[:])

            # repeat g columns 4x: col p = j*BL + b  ->  g[:, b]
            gT4 = sb.tile([128, 128], fp32)
            nc.vector.tensor_copy(
                gT4[:].rearrange("p (j b) -> p j b", j=J),
                gT[:].unsqueeze(1).broadcast_to([128, J, BL]))

            # w37 [128,7]: 0:3 w3 | 3 cn | 4:6 wd | 6 cd   (per partition's b)
            w37_p = psq.tile([128, 7], fp32, tag="pqg")
            nc.tensor.matmul(w37_p[:], lhsT=gT4[:], rhs=w7T[:],
                             start=True, stop=True)
            w37 = sb.tile([128, 7], fp32)
            nc.vector.tensor_copy(w37[:], w37_p[:])
            biasN = sb.tile([128, 1], fp32)
            nc.vector.tensor_scalar(biasN[:], w37[:, 3:4], NORM,
                                    MASK_BIG * NORM, op0=Alu.mult,
                                    op1=Alu.subtract)
            w33b = sb.tile([128, 3], bf16)
            nc.vector.tensor_copy(w33b[:], w37[:, 0:3])

            # ---------------- logits stream ----------------
            tmp = sb.tile([128, NF * 3], bf16)
            for k in range(KC):
                ksl = slice(k * NFC * 3, (k + 1) * NFC * 3)
                nc.vector.tensor_tensor(
                    tmp[:, ksl].rearrange("p (f c) -> p f c", c=3),
                    x[:, ksl].rearrange("p (f c) -> p f c", c=3),
                    w33b[:].unsqueeze(1).broadcast_to([128, NFC, 3]),
                    op=Alu.mult)
            L = sb.tile([128, NF], fp32)
            nc.vector.tensor_reduce(
                L[:], tmp[:].rearrange("p (f c) -> p f c", c=3),
                axis=X, op=Alu.add)

            # depot logit at n=0 (rows 0:BL are the j=0 block, b-ordered):
            # L[b,0] = xd.wd + cd - cn  (cn re-added by the exp bias)
            cdmn = sb.tile([BL, 1], fp32)
            nc.vector.tensor_tensor(cdmn[:], w37[0:BL, 6:7], w37[0:BL, 3:4],
                                    op=Alu.subtract)
            dlm = sb.tile([BL, 2], fp32)
            nc.vector.tensor_tensor(dlm[:], bp_c[:, 4:6], w37[0:BL, 4:6],
                                    op=Alu.mult)
            dlr = sb.tile([BL, 1], fp32)
            nc.vector.tensor_reduce(dlr[:], dlm[:], axis=X, op=Alu.add)
            nc.vector.tensor_tensor(L[0:BL, 0:1], dlr[:], cdmn[:], op=Alu.add)

            # L += MASK_BIG*mask  (the -MASK_BIG constant lives in the exp bias)
            nc.vector.scalar_tensor_tensor(L[:], mku[:], MASK_BIG, L[:],
                                           op0=Alu.mult, op1=Alu.add)

            # E = exp(NORM*L + NORM*cn); accum_out -> row sums (f32)
            E = sb.tile([128, NF], bf16)
            sjact = sb.tile([128, 1], fp32)
            nc.scalar.activation(E[:], L[:], Act.Exp, bias=biasN[:],
                                 scale=NORM, accum_out=sjact[:])
            s3S = sb.tile([128, 4], fp32)
            nc.vector.tensor_copy(s3S[:, 3:4], sjact[:])

            # s3 partials: s3S[:,0:3] = sum_f E * x
            tmp2 = sb.tile([128, NF * 3], bf16)
            nc.vector.tensor_tensor(
                tmp2[:].rearrange("p (f c) -> p f c", c=3),
                x[:].rearrange("p (f c) -> p f c", c=3),
                E[:].unsqueeze(2).broadcast_to([128, NF, 3]), op=Alu.mult)
            nc.vector.tensor_reduce(
                s3S[:, 0:3], tmp2[:].rearrange("p (f c) -> p c f", c=3),
                axis=X, op=Alu.add)

            # cross-j reduction: S[b,:] = sum_j s3S[j*BL+b,:]
            s3b_p = ps.tile([BL, 4], fp32, tag="pt")
            nc.tensor.matmul(s3b_p[:], lhsT=rep_eye[:], rhs=s3S[:],
                             start=True, stop=True)

            recipS = sb.tile([BL, 1], fp32)
            nc.vector.reciprocal(recipS[:], s3b_p[:, 3:4])

            # urhs [32,7]: 0:3 s3 | 3 1-a0 | 4:6 a0*xd | 6 a0
            urhs = sb.tile([BL, 7], fp32)
            nc.vector.tensor_scalar(urhs[:, 6:7], E[0:BL, 0:1], recipS[:],
                                    None, op0=Alu.mult)
            t1 = sb.tile([BL, 3], fp32)
            nc.vector.tensor_scalar(t1[:], s3b_p[:, 0:3], recipS[:], None,
                                    op0=Alu.mult)
            t2 = sb.tile([BL, 3], fp32)
            nc.vector.tensor_scalar(t2[:], bp_c[:, 4:7], urhs[:, 6:7], None,
                                    op0=Alu.mult)
            nc.vector.tensor_tensor(urhs[:, 0:3], t1[:], t2[:],
                                    op=Alu.subtract)
            nc.vector.tensor_scalar(urhs[:, 3:4], urhs[:, 6:7], -1.0, 1.0,
                                    op0=Alu.mult, op1=Alu.add)
            nc.vector.tensor_copy(urhs[:, 4:6], t2[:, 0:2])

            urT_p = ps.tile([7, BL], fp32, tag="pt")
            nc.tensor.transpose(urT_p[:], urhs[:], idb)
            urT = sb.tile([7, BL], fp32)
            nc.vector.tensor_copy(urT[:], urT_p[:])

            u_p = psq.tile([128, BL], fp32, tag="pqg")
            nc.tensor.matmul(u_p[:], lhsT=w7c, rhs=urT[:],
                             start=True, stop=True)
            u_sb = sb.tile([128, BL], fp32)
            nc.vector.tensor_copy(u_sb[:], u_p[:])

            hT_p = psq.tile([128, BL], fp32, tag="pqg")
            nc.tensor.matmul(hT_p[:], lhsT=wv2_c, rhs=u_sb[:],
                             start=True, stop=False)
            nc.tensor.matmul(hT_p[:], lhsT=wv1_c, rhs=currT[:],
                             start=False, stop=True)
            hT = sb.tile([128, BL], fp32)
            nc.vector.tensor_copy(hT[:], hT_p[:])
            h_p = ps.tile([BL, 128], fp32, tag="pt")
            nc.tensor.transpose(h_p[:], hT[:], ident[:])
            h_sb = sb.tile([BL, 128], fp32)
            nc.vector.tensor_copy(h_sb[:], h_p[:])
            nc.sync.dma_start(out[:], h_sb[:])

    if finalize:
        nc.finalize()
    return nc


def _shard_inputs(node_feats, state, W_node, b_node, W_depot, b_depot,
                  W_state, b_state, w_q, w_k, w_v, curr_node_id,
                  next_node_id, mask):
    import ml_dtypes

    f32 = np.float32
    node_feats = np.ascontiguousarray(node_feats, dtype=f32)
    nf_bf16 = node_feats.astype(ml_dtypes.bfloat16)
    state = np.ascontiguousarray(state, dtype=f32)
    mask_u8 = np.ascontiguousarray(mask).astype(np.uint8)
    ids = np.stack([np.asarray(curr_node_id), np.asarray(next_node_id)],
                   axis=1).astype(np.int32)

    W_node = np.asarray(W_node, dtype=f32)
    b_node = np.asarray(b_node, dtype=f32)
    W_depot = np.asarray(W_depot, dtype=f32)
    b_depot = np.asarray(b_depot, dtype=f32)
    W_state = np.asarray(W_state, dtype=f32)
    b_state = np.asarray(b_state, dtype=f32)
    w_q = np.asarray(w_q, dtype=f32)
    w_k = np.asarray(w_k, dtype=f32)
    w_v = np.asarray(w_v, dtype=f32)

    w7s = np.concatenate([W_node, b_node[None, :], W_depot,
                          b_depot[None, :]], axis=0)           # [7,128]
    wpack = np.zeros((7, 512), f32)
    wpack[:, 0:128] = w7s
    wpack[0:4, 128:256] = W_state
    wpack[0:3, 256:384] = W_node
    wpack[0:2, 384:512] = W_depot
    wbig = np.zeros((128, WBIG_COLS), f32)
    wbig[:, 0:128] = w_q[0:128]
    wbig[:, 128:256] = w_q[128:256]
    wbig[:, 256:384] = w_q[256:384]
    wbig[:, 384:512] = w_k[128:256]
    wbig[:, 512:640] = w_v[0:128]
    wbig[:, 640:768] = w_v[128:256]
    wbig[:, 768] = b_node
    wbig[:, 769] = b_depot
    wbig[:, 770] = b_state

    shared = {
        "wpack": np.ascontiguousarray(wpack),
        "wbig": np.ascontiguousarray(wbig),
    }
    in_maps = []
    for i in range(NCORES):
        s = slice(i * BL, (i + 1) * BL)
        m = dict(shared)
        m["node_feats"] = np.ascontiguousarray(nf_bf16[s])
        bp = np.concatenate([state[s], node_feats[s, 0, :]], axis=1)
        m["bpack"] = np.ascontiguousarray(bp)
        m["ids"] = np.ascontiguousarray(ids[s])
        m["mask_u8"] = np.ascontiguousarray(mask_u8[s])
        in_maps.append(m)
    return in_maps


def _run(inputs, trace=False):
    from concourse.bass_utils import run_bass_kernel_spmd

    if "nc" not in _CACHE:
        _CACHE["nc"] = _build()
    nc = _CACHE["nc"]
    in_maps = _shard_inputs(**inputs)
    res = run_bass_kernel_spmd(nc, in_maps, core_ids=list(range(NCORES)),
                               trace=trace)
    full = np.concatenate([r["out"] for r in res.results], axis=0)
    return full, res


def kernel(**inputs):
    full, _ = _run(inputs, trace=False)
    return full


# revision 30
# speedup vs baseline: 1.5509x; 1.1152x over previous
"""Trainium2 Bass kernel for nn_AttentionGraphEncoder (gnn_message_passing).

Math restructure (exact, not approximate):
  The reference computes, per batch b:
    nf[n]   = embed(node_feats[b,n])                  # [N, E] affine in x
    k[n]    = curr_emb @ Wk1 + nf[n] @ Wk2
    logits  = NORM * q . k[n]  -> softmax (masked)
    v[n]    = curr_emb @ Wv1 + nf[n] @ Wv2
    h       = sum_n attn[n] v[n]
  Since q.k[n] = const + nf[n].(Wk2 @ q) and nf[n] is affine in the raw
  3-dim node coordinates, the logits reduce to x[n] . w3(b) + const with
  w3(b) = W_node @ Wk2 @ q(b)  (a 3-vector per batch).  Likewise
    h = curr_emb @ Wv1 + (a0*depot_emb + (1-a0)*b_node + s3 @ W_node) @ Wv2
  with s3 = sum_{n>=1} attn[n] x[n] (a 3-vector) and a0 = attn[0].
  So the big [N,2E]@[2E,E] matmuls disappear; the kernel streams
  node_feats ([B,N,3], shipped bf16) once through the DVE and does tiny
  matmuls.

Sharding: pure data parallel, batch 256 -> 32 per core across 8 cores.

Perf notes:
  - DMA issue blocks the issuing engine's queue for the transfer, so the
    x stream is split across the Scalar and Tensor queues while Sync
    carries the (packed) weights; ids goes absolutely first (it heads
    the longest dependency chain: gather -> q -> w3 -> logits).
  - Small weights are packed host-side into three tensors so the Sync
    queue issues 5 DMAs instead of ~15.
  - The node stream is bf16 (halves HBM traffic, 2x DVE rate); all
    reductions accumulate in f32.
"""

import math

import numpy as np

B, N, NODE_DIM, STATE_DIM, EMB = 256, 2048, 3, 4, 128
NCORES = 8
BL = B // NCORES          # 32 batch elements per core
J = 4                     # node-chunks per batch -> 128 partitions (j*BL + b)
NF = N // J               # 512 nodes per partition row
KC = 4                    # free-dim DMA/compute chunks of the x stream
NFC = NF // KC            # 128 nodes per chunk
NORM = 1.0 / math.sqrt(EMB)
MASK_BIG = 400.0          # pre-NORM additive mask magnitude (400*NORM ~ 35)
WBIG_COLS = 3 * EMB + EMB + 8             # wq | Wv1 | W7kT | pad = 520

_CACHE = {}


def _build(finalize=True):
    import concourse.bacc as bacc
    import concourse.bass as bass
    import concourse.mybir as mybir
    import concourse.tile as tile
    from concourse.masks import make_identity

    fp32 = mybir.dt.float32
    bf16 = mybir.dt.bfloat16
    i32 = mybir.dt.int32
    u8 = mybir.dt.uint8
    Alu = mybir.AluOpType
    Act = mybir.ActivationFunctionType
    X = mybir.AxisListType.X

    nc = bacc.Bacc("TRN2")

    nfd = nc.dram_tensor("node_feats", [BL, N, NODE_DIM], bf16,
                         kind="ExternalInput")
    # bpack: state(4) | 1.0 | x0(3)  per batch row, f32
    bpk = nc.dram_tensor("bpack", [BL, 8], fp32, kind="ExternalInput")
    # wpack [7, 384]: w7s | [W_state; b_state] (rows 0:5) | W7v = w7s @ Wv2
    wpk = nc.dram_tensor("wpack", [7, 384], fp32, kind="ExternalInput")
    # wbig [128, 520]: wq(384) | Wv1(128) | W7kT = (w7s @ Wk2)^T (7) | pad
    wbg = nc.dram_tensor("wbig", [128, WBIG_COLS], fp32, kind="ExternalInput")
    ids = nc.dram_tensor("ids", [BL, 2], i32, kind="ExternalInput")
    mk = nc.dram_tensor("mask_u8", [BL, N], u8, kind="ExternalInput")
    out = nc.dram_tensor("out", [BL, EMB], fp32, kind="ExternalOutput")

    with tile.TileContext(nc, pool_alloc_mode="queue") as tc:
        with (
            tc.tile_pool(name="sb", bufs=1) as sb,
            tc.tile_pool(name="ps", bufs=3, space="PSUM") as ps,
            tc.tile_pool(name="pse", bufs=1, space="PSUM") as pse,
            tc.tile_pool(name="psq", bufs=2, space="PSUM") as psq,
        ):
            # ------- gpsimd constants (all before the PE warm-up) -------
            ident = sb.tile([128, 128], fp32)
            make_identity(nc, ident[:])
            iota_p = sb.tile([BL, 1], i32)
            nc.gpsimd.iota(iota_p[:], pattern=[[0, 1]], base=0,
                           channel_multiplier=N)
            # rep_eye[p, y] = 1 iff p % BL == y  (cross-j reduce as a matmul)
            rep_eye = sb.tile([128, BL], fp32)
            nc.gpsimd.memset(rep_eye[:], 0.0)
            for j in range(J):
                nc.gpsimd.affine_select(
                    out=rep_eye[:], in_=rep_eye[:],
                    compare_op=Alu.not_equal, fill=1.0,
                    base=-BL * j, pattern=[[-1, BL]], channel_multiplier=1)
            # PE warm-up: one op depending on the LAST gpsimd constant so
            # later PE ops see all Pool ticks as observed.
            junk_p = ps.tile([1, 1], fp32, tag="pt")
            nc.tensor.matmul(junk_p[:], lhsT=rep_eye[:, 0:1],
                             rhs=ident[:, 0:1], start=True, stop=True)

            # ------------------- input DMAs -------------------
            # Sync queue, most-critical first.
            ids_sb = sb.tile([BL, 2], i32)
            nc.sync.dma_start(ids_sb[:], ids[:])
            bp_sb = sb.tile([BL, 8], fp32)
            nc.sync.dma_start(bp_sb[:], bpk[:])
            wp_sb = sb.tile([7, 384], fp32)
            nc.sync.dma_start(wp_sb[:], wpk[:])
            wb_sb = sb.tile([128, WBIG_COLS], fp32)
            nc.sync.dma_start(wb_sb[:], wbg[:])
            # node stream (bf16) split across the Scalar + Sync queues
            x = sb.tile([128, NF * 3], bf16)
            xview = (nfd[:].rearrange("b (j f) c -> b j (f c)", j=J)
                     .transpose([1, 0, 2]))                 # [J, BL, NF*3]
            for k in range(KC):
                ksl = slice(k * NFC * 3, (k + 1) * NFC * 3)
                eng = nc.scalar if k % 2 == 0 else nc.sync
                eng.dma_start(x[:, ksl], xview[:, :, ksl])
            mku = sb.tile([128, NF], u8)
            mview = (mk[:].rearrange("b (j f) -> b j f", j=J)
                     .transpose([1, 0, 2]))                 # [J, BL, NF]
            nc.scalar.dma_start(mku[:], mview[:])

            # Bacc splits excess waits onto event semaphores, so engines can
            # consume the DMA'd tiles directly (no funnel copies needed).
            ids_c = ids_sb
            bp_c = bp_sb
            w7c = wp_sb[:, 0:128]          # [7,128]  [Wn; bn; Wd; bd]
            ws5_c = wp_sb[0:5, 128:256]    # [5,128]  [Ws; bs]
            w7v_c = wp_sb[:, 256:384]      # [7,128]  w7s @ Wv2
            wq_c = wb_sb[:, 0:3 * EMB]
            wv1_c = wb_sb[:, 3 * EMB:4 * EMB]
            w7kT_c = wb_sb[:, 4 * EMB:4 * EMB + 7]   # [128,7]

            # ---------------- gathers ----------------
            offs = sb.tile([BL, 2], i32)
            nc.vector.tensor_tensor(offs[:], ids_c[:],
                                    iota_p[:].to_broadcast([BL, 2]),
                                    op=Alu.add)
            nfr = nfd[:].rearrange("b n c -> (b n) c")
            xc3 = sb.tile([BL, NODE_DIM], bf16)
            nc.gpsimd.indirect_dma_start(
                out=xc3[:], out_offset=None, in_=nfr,
                in_offset=bass.IndirectOffsetOnAxis(ap=offs[:, 0:1], axis=0))
            xn3 = sb.tile([BL, NODE_DIM], bf16)
            nc.gpsimd.indirect_dma_start(
                out=xn3[:], out_offset=None, in_=nfr,
                in_offset=bass.IndirectOffsetOnAxis(ap=offs[:, 1:2], axis=0))

            # extended blended inputs: inp7 = [(1-z)*xg | 1-z | z*xd | z]
            # so that inp7 @ w7s == blended node/depot embedding (incl bias)
            isz2 = sb.tile([BL, 2], fp32)
            nc.vector.tensor_scalar(isz2[:], ids_c[:], 0, None,
                                    op0=Alu.is_equal)
            omz = sb.tile([BL, 2], fp32)
            nc.vector.tensor_scalar(omz[:], isz2[:], -1.0, 1.0,
                                    op0=Alu.mult, op1=Alu.add)
            c7 = sb.tile([BL, 7], fp32)
            nc.vector.tensor_scalar(c7[:, 0:3], xc3[:], omz[:, 0:1], None,
                                    op0=Alu.mult)
            nc.vector.tensor_copy(c7[:, 3:4], omz[:, 0:1])
            nc.vector.tensor_scalar(c7[:, 4:6], bp_c[:, 5:7], isz2[:, 0:1],
                                    None, op0=Alu.mult)
            nc.vector.tensor_copy(c7[:, 6:7], isz2[:, 0:1])
            n7 = sb.tile([BL, 7], fp32)
            nc.vector.tensor_scalar(n7[:, 0:3], xn3[:], omz[:, 1:2], None,
                                    op0=Alu.mult)
            nc.vector.tensor_copy(n7[:, 3:4], omz[:, 1:2])
            nc.vector.tensor_scalar(n7[:, 4:6], bp_c[:, 5:7], isz2[:, 1:2],
                                    None, op0=Alu.mult)
            nc.vector.tensor_copy(n7[:, 6:7], isz2[:, 1:2])

            # transpose inputs to [feat, b]: one PSUM tile, three col blocks
            idb = ident[0:BL, 0:BL]
            t3_p = ps.tile([7, 3 * BL], fp32, tag="pt")
            nc.tensor.transpose(t3_p[0:7, 0:BL], c7[:], idb)
            nc.tensor.transpose(t3_p[0:7, BL:2 * BL], n7[:], idb)
            nc.tensor.transpose(t3_p[0:5, 2 * BL:3 * BL], bp_c[:, 0:5], idb)
            t3in = sb.tile([7, 3 * BL], fp32)
            nc.vector.tensor_copy(t3in[0:7, 0:2 * BL], t3_p[0:7, 0:2 * BL])
            nc.vector.tensor_copy(t3in[0:5, 2 * BL:3 * BL],
                                  t3_p[0:5, 2 * BL:3 * BL])

            # ---------- embeddings (transposed [E, b]) ----------
            pemb = pse.tile([128, 3 * BL], fp32, tag="pemb")
            nc.tensor.matmul(pemb[:, 0:BL], lhsT=w7c, rhs=t3in[0:7, 0:BL],
                             start=True, stop=True)
            nc.tensor.matmul(pemb[:, BL:2 * BL], lhsT=w7c,
                             rhs=t3in[0:7, BL:2 * BL], start=True, stop=True)
            nc.tensor.matmul(pemb[:, 2 * BL:3 * BL], lhsT=ws5_c,
                             rhs=t3in[0:5, 2 * BL:3 * BL],
                             start=True, stop=True)
            t3 = sb.tile([128, 3 * BL], fp32)
            nc.vector.tensor_copy(t3[:], pemb[:])
            currT = t3[:, 0:BL]

            # ---------------- q, w37 ----------------
            qT_p = psq.tile([128, BL], fp32, tag="pqg")
            nc.tensor.matmul(qT_p[:], lhsT=wq_c[:, 0:EMB], rhs=t3[:, 0:BL],
                             start=True, stop=False)
            nc.tensor.matmul(qT_p[:], lhsT=wq_c[:, EMB:2 * EMB],
                             rhs=t3[:, BL:2 * BL], start=False, stop=False)
            nc.tensor.matmul(qT_p[:], lhsT=wq_c[:, 2 * EMB:3 * EMB],
                             rhs=t3[:, 2 * BL:3 * BL], start=False, stop=True)
            qT = sb.tile([128, BL], fp32)
            nc.vector.tensor_copy(qT[:], qT_p[:])

            # repeat q columns 4x: col p = j*BL + b  ->  q[:, b]
            qT4 = sb.tile([128, 128], fp32)
            nc.vector.tensor_copy(
                qT4[:].rearrange("p (j b) -> p j b", j=J),
                qT[:].unsqueeze(1).broadcast_to([128, J, BL]))

            # w37 [128,7]: 0:3 w3 | 3 cn | 4:6 wd | 6 cd   (per partition's b)
            w37_p = psq.tile([128, 7], fp32, tag="pqg")
            nc.tensor.matmul(w37_p[:], lhsT=qT4[:], rhs=w7kT_c,
                             start=True, stop=True)
            w37 = sb.tile([128, 7], fp32)
            nc.vector.tensor_copy(w37[:], w37_p[:])
            biasN = sb.tile([128, 1], fp32)
            nc.vector.tensor_scalar(biasN[:], w37[:, 3:4], NORM,
                                    MASK_BIG * NORM, op0=Alu.mult,
                                    op1=Alu.subtract)
            w33b = sb.tile([128, 3], bf16)
            nc.vector.tensor_copy(w33b[:], w37[:, 0:3])

            # ---------------- logits stream ----------------
            tmp = sb.tile([128, NF * 3], bf16)
            for k in range(KC):
                ksl = slice(k * NFC * 3, (k + 1) * NFC * 3)
                nc.vector.tensor_tensor(
                    tmp[:, ksl].rearrange("p (f c) -> p f c", c=3),
                    x[:, ksl].rearrange("p (f c) -> p f c", c=3),
                    w33b[:].unsqueeze(1).broadcast_to([128, NFC, 3]),
                    op=Alu.mult)
            L = sb.tile([128, NF], fp32)
            nc.vector.tensor_reduce(
                L[:], tmp[:].rearrange("p (f c) -> p f c", c=3),
                axis=X, op=Alu.add)

            # depot logit at n=0 (rows 0:BL are the j=0 block, b-ordered):
            # L[b,0] = xd.wd + cd - cn  (cn re-added by the exp bias)
            cdmn = sb.tile([BL, 1], fp32)
            nc.vector.tensor_tensor(cdmn[:], w37[0:BL, 6:7], w37[0:BL, 3:4],
                                    op=Alu.subtract)
            dlm = sb.tile([BL, 2], fp32)
            nc.vector.tensor_tensor(dlm[:], bp_c[:, 5:7], w37[0:BL, 4:6],
                                    op=Alu.mult)
            dlr = sb.tile([BL, 1], fp32)
            nc.vector.tensor_reduce(dlr[:], dlm[:], axis=X, op=Alu.add)
            nc.vector.tensor_tensor(L[0:BL, 0:1], dlr[:], cdmn[:], op=Alu.add)

            # L += MASK_BIG*mask  (the -MASK_BIG constant lives in the exp bias)
            nc.vector.scalar_tensor_tensor(L[:], mku[:], MASK_BIG, L[:],
                                           op0=Alu.mult, op1=Alu.add)

            # E = exp(NORM*L + NORM*cn); accum_out -> row sums (f32)
            E = sb.tile([128, NF], bf16)
            sjact = sb.tile([128, 1], fp32)
            nc.scalar.activation(E[:], L[:], Act.Exp, bias=biasN[:],
                                 scale=NORM, accum_out=sjact[:])
            s3S = sb.tile([128, 4], fp32)
            nc.vector.tensor_copy(s3S[:, 3:4], sjact[:])

            # s3 partials: s3S[:,0:3] = sum_f E * x
            tmp2 = sb.tile([128, NF * 3], bf16)
            nc.vector.tensor_tensor(
                tmp2[:].rearrange("p (f c) -> p f c", c=3),
                x[:].rearrange("p (f c) -> p f c", c=3),
                E[:].unsqueeze(2).broadcast_to([128, NF, 3]), op=Alu.mult)
            nc.vector.tensor_reduce(
                s3S[:, 0:3], tmp2[:].rearrange("p (f c) -> p c f", c=3),
                axis=X, op=Alu.add)

            # cross-j reduction: S[b,:] = sum_j s3S[j*BL+b,:]
            s3b_p = ps.tile([BL, 4], fp32, tag="pt")
            nc.tensor.matmul(s3b_p[:], lhsT=rep_eye[:], rhs=s3S[:],
                             start=True, stop=True)

            recipS = sb.tile([BL, 1], fp32)
            nc.vector.reciprocal(recipS[:], s3b_p[:, 3:4])

            # urhs [32,7]: 0:3 s3 | 3 1-a0 | 4:6 a0*xd | 6 a0
            urhs = sb.tile([BL, 7], fp32)
            nc.vector.tensor_scalar(urhs[:, 6:7], E[0:BL, 0:1], recipS[:],
                                    None, op0=Alu.mult)
            t1 = sb.tile([BL, 3], fp32)
            nc.vector.tensor_scalar(t1[:], s3b_p[:, 0:3], recipS[:], None,
                                    op0=Alu.mult)
            t2 = sb.tile([BL, 3], fp32)
            nc.vector.tensor_scalar(t2[:], bp_c[:, 5:8], urhs[:, 6:7], None,
                                    op0=Alu.mult)
            nc.vector.tensor_tensor(urhs[:, 0:3], t1[:], t2[:],
                                    op=Alu.subtract)
            nc.vector.tensor_scalar(urhs[:, 3:4], urhs[:, 6:7], -1.0, 1.0,
                                    op0=Alu.mult, op1=Alu.add)
            nc.vector.tensor_copy(urhs[:, 4:6], t2[:, 0:2])

            urT_p = ps.tile([7, BL], fp32, tag="pt")
            nc.tensor.transpose(urT_p[:], urhs[:], idb)
            urT = sb.tile([7, BL], fp32)
            nc.vector.tensor_copy(urT[:], urT_p[:])

            # h [b, e] directly: curr_emb @ Wv1 + urhs @ W7v
            h_p = ps.tile([BL, EMB], fp32, tag="pt")
            nc.tensor.matmul(h_p[:], lhsT=currT, rhs=wv1_c,
                             start=True, stop=False)
            nc.tensor.matmul(h_p[:], lhsT=urT[:], rhs=w7v_c,
                             start=False, stop=True)
            h_sb = sb.tile([BL, EMB], fp32)
            nc.vector.tensor_copy(h_sb[:], h_p[:])
            nc.sync.dma_start(out[:], h_sb[:])

    if finalize:
        nc.finalize()
    return nc


def _shard_inputs(node_feats, state, W_node, b_node, W_depot, b_depot,
                  W_state, b_state, w_q, w_k, w_v, curr_node_id,
                  next_node_id, mask):
    import ml_dtypes

    f32 = np.float32
    node_feats = np.ascontiguousarray(node_feats, dtype=f32)
    nf_bf16 = node_feats.astype(ml_dtypes.bfloat16)
    state = np.ascontiguousarray(state, dtype=f32)
    mask_u8 = np.ascontiguousarray(mask).astype(np.uint8)
    ids = np.stack([np.asarray(curr_node_id), np.asarray(next_node_id)],
                   axis=1).astype(np.int32)

    W_node = np.asarray(W_node, dtype=f32)
    b_node = np.asarray(b_node, dtype=f32)
    W_depot = np.asarray(W_depot, dtype=f32)
    b_depot = np.asarray(b_depot, dtype=f32)
    W_state = np.asarray(W_state, dtype=f32)
    b_state = np.asarray(b_state, dtype=f32)
    w_q = np.asarray(w_q, dtype=f32)
    w_k = np.asarray(w_k, dtype=f32)
    w_v = np.asarray(w_v, dtype=f32)

    w7s = np.concatenate([W_node, b_node[None, :], W_depot,
                          b_depot[None, :]], axis=0)           # [7,128]
    Wk2 = w_k[128:256]
    Wv2 = w_v[128:256]
    W7k = w7s @ Wk2                                            # [7,128]
    W7v = w7s @ Wv2                                            # [7,128]
    wpack = np.zeros((7, 384), f32)
    wpack[:, 0:128] = w7s
    wpack[0:4, 128:256] = W_state
    wpack[4, 128:256] = b_state
    wpack[:, 256:384] = W7v
    wbig = np.zeros((128, WBIG_COLS), f32)
    wbig[:, 0:128] = w_q[0:128]
    wbig[:, 128:256] = w_q[128:256]
    wbig[:, 256:384] = w_q[256:384]
    wbig[:, 384:512] = w_v[0:128]
    wbig[:, 512:519] = W7k.T

    shared = {
        "wpack": np.ascontiguousarray(wpack),
        "wbig": np.ascontiguousarray(wbig),
    }
    in_maps = []
    for i in range(NCORES):
        s = slice(i * BL, (i + 1) * BL)
        m = dict(shared)
        m["node_feats"] = np.ascontiguousarray(nf_bf16[s])
        bp = np.concatenate([state[s], np.ones((BL, 1), f32),
                             node_feats[s, 0, :]], axis=1)
        m["bpack"] = np.ascontiguousarray(bp)
        m["ids"] = np.ascontiguousarray(ids[s])
        m["mask_u8"] = np.ascontiguousarray(mask_u8[s])
        in_maps.append(m)
    return in_maps


def _run(inputs, trace=False):
    from concourse.bass_utils import run_bass_kernel_spmd

    if "nc" not in _CACHE:
        _CACHE["nc"] = _build()
    nc = _CACHE["nc"]
    in_maps = _shard_inputs(**inputs)
    res = run_bass_kernel_spmd(nc, in_maps, core_ids=list(range(NCORES)),
                               trace=trace)
    full = np.concatenate([r["out"] for r in res.results], axis=0)
    return full, res


def kernel(**inputs):
    full, _ = _run(inputs, trace=False)
    return full


# revision 31
# speedup vs baseline: 1.6238x; 1.0470x over previous
"""Trainium2 Bass kernel for nn_AttentionGraphEncoder (gnn_message_passing).

Math restructure (exact, not approximate):
  Per batch b the reference computes masked attention over N=2048 nodes
  whose keys/values are affine in the raw 3-dim node coordinates, so
    logits[n] = x[n] . w3(b) + const,   w3(b) = (w7s @ Wk2) rows @ q(b)
    h = curr_emb @ Wv1 + urhs(b) @ (w7s @ Wv2)
  with urhs = [s3 | 1-a0 | a0*xd | a0], s3 = sum_{n>=1} attn[n] x[n].
  The big [N,2E]@[2E,E] matmuls disappear; the kernel streams node_feats
  (bf16) once through the DVE and does tiny matmuls.  Weight-only
  products (w7s@Wk2, w7s@Wv2) are folded on the host.

Sharding: pure data parallel, batch 256 -> 32 per core across 8 cores.

Perf notes:
  - DMA issue blocks the issuing engine queue, so the x stream is split
    across the Scalar and Sync queues; ids goes first (it heads the
    longest chain: gather -> q -> w3 -> logits).
  - Node/depot blending happens in the gathered input vector (inp7 =
    [(1-z)xg | 1-z | z*xd | z]) so one [7,128] matmul produces the
    blended embedding including biases.
  - All small matmuls run bf16 (fp32 PE mode is 2-pass); logits/sums
    accumulate f32.
  - L and s3 use per-partition-scalar chained ops (scalar_tensor_tensor
    with f32 accumulate) instead of materialize+reduce.
"""

import math

import numpy as np

B, N, NODE_DIM, STATE_DIM, EMB = 256, 2048, 3, 4, 128
NCORES = 8
BL = B // NCORES          # 32 batch elements per core
J = 4                     # node-chunks per batch -> 128 partitions (j*BL + b)
NF = N // J               # 512 nodes per partition row
KC = 4                    # free-dim DMA chunks of the x stream
NFC = NF // KC
NORM = 1.0 / math.sqrt(EMB)
MASK_BIG = 400.0          # pre-NORM additive mask magnitude (400*NORM ~ 35)
WBIG_COLS = 3 * EMB + EMB + 8             # wq | Wv1 | W7kT | pad = 520

_CACHE = {}


def _build(finalize=True):
    import concourse.bacc as bacc
    import concourse.bass as bass
    import concourse.mybir as mybir
    import concourse.tile as tile
    from concourse.masks import make_identity

    fp32 = mybir.dt.float32
    bf16 = mybir.dt.bfloat16
    i32 = mybir.dt.int32
    u8 = mybir.dt.uint8
    Alu = mybir.AluOpType
    Act = mybir.ActivationFunctionType
    X = mybir.AxisListType.X

    nc = bacc.Bacc("TRN2")

    nfd = nc.dram_tensor("node_feats", [BL, N, NODE_DIM], bf16,
                         kind="ExternalInput")
    # bpack: state(4) | 1.0 | x0(3)  per batch row, f32
    bpk = nc.dram_tensor("bpack", [BL, 8], fp32, kind="ExternalInput")
    # wpack [7, 384] bf16: w7s | [W_state; b_state] (rows 0:5) | W7v
    wpk = nc.dram_tensor("wpack", [7, 384], bf16, kind="ExternalInput")
    # wbig [128, 520] bf16: wq(384) | Wv1(128) | W7kT(7) | pad
    wbg = nc.dram_tensor("wbig", [128, WBIG_COLS], bf16, kind="ExternalInput")
    ids = nc.dram_tensor("ids", [BL, 2], i32, kind="ExternalInput")
    mk = nc.dram_tensor("mask_u8", [BL, N], u8, kind="ExternalInput")
    out = nc.dram_tensor("out", [BL, EMB], fp32, kind="ExternalOutput")

    with tile.TileContext(nc, pool_alloc_mode="queue") as tc:
        with (
            tc.tile_pool(name="sb", bufs=1) as sb,
            tc.tile_pool(name="ps", bufs=3, space="PSUM") as ps,
            tc.tile_pool(name="pse", bufs=1, space="PSUM") as pse,
            tc.tile_pool(name="psq", bufs=2, space="PSUM") as psq,
        ):
            # ------- gpsimd constants (all before the PE warm-up) -------
            iota_p = sb.tile([BL, 1], i32)
            nc.gpsimd.iota(iota_p[:], pattern=[[0, 1]], base=0,
                           channel_multiplier=N)
            identb = sb.tile([BL, BL], bf16)
            make_identity(nc, identb[:])
            # rep_eye[p, y] = 1 iff p % BL == y  (cross-j reduce as a matmul)
            rep_eye = sb.tile([128, BL], fp32)
            nc.gpsimd.memset(rep_eye[:], 0.0)
            for j in range(J):
                nc.gpsimd.affine_select(
                    out=rep_eye[:], in_=rep_eye[:],
                    compare_op=Alu.not_equal, fill=1.0,
                    base=-BL * j, pattern=[[-1, BL]], channel_multiplier=1)
            # PE warm-up: one op depending on the LAST gpsimd constant so
            # later PE ops see all Pool ticks as observed.
            junk_p = ps.tile([1, 1], fp32, tag="pt")
            nc.tensor.matmul(junk_p[:], lhsT=rep_eye[:, 0:1],
                             rhs=rep_eye[:, 0:1], start=True, stop=True)

            # ------------------- input DMAs -------------------
            ids_sb = sb.tile([BL, 2], i32)
            nc.sync.dma_start(ids_sb[:], ids[:])
            bp_sb = sb.tile([BL, 8], fp32)
            nc.sync.dma_start(bp_sb[:], bpk[:])
            wp_sb = sb.tile([7, 384], bf16)
            nc.sync.dma_start(wp_sb[:], wpk[:])
            wb_sb = sb.tile([128, WBIG_COLS], bf16)
            nc.sync.dma_start(wb_sb[:], wbg[:])
            x = sb.tile([128, NF * 3], bf16)
            xview = (nfd[:].rearrange("b (j f) c -> b j (f c)", j=J)
                     .transpose([1, 0, 2]))                 # [J, BL, NF*3]
            for k in range(KC):
                ksl = slice(k * NFC * 3, (k + 1) * NFC * 3)
                eng = nc.scalar if k % 2 == 0 else nc.sync
                eng.dma_start(x[:, ksl], xview[:, :, ksl])
            mku = sb.tile([128, NF], u8)
            mview = (mk[:].rearrange("b (j f) -> b j f", j=J)
                     .transpose([1, 0, 2]))                 # [J, BL, NF]
            nc.scalar.dma_start(mku[:], mview[:])

            ids_c = ids_sb
            bp_c = bp_sb
            w7c = wp_sb[:, 0:128]          # [7,128]  [Wn; bn; Wd; bd]
            ws5_c = wp_sb[0:5, 128:256]    # [5,128]  [Ws; bs]
            w7v_c = wp_sb[:, 256:384]      # [7,128]  w7s @ Wv2
            wq_c = wb_sb[:, 0:3 * EMB]
            wv1_c = wb_sb[:, 3 * EMB:4 * EMB]
            w7kT_c = wb_sb[:, 4 * EMB:4 * EMB + 7]   # [128,7]

            # ---------------- gathers ----------------
            offs = sb.tile([BL, 2], i32)
            nc.vector.tensor_tensor(offs[:], ids_c[:],
                                    iota_p[:].to_broadcast([BL, 2]),
                                    op=Alu.add)
            nfr = nfd[:].rearrange("b n c -> (b n) c")
            xc3 = sb.tile([BL, NODE_DIM], bf16)
            nc.gpsimd.indirect_dma_start(
                out=xc3[:], out_offset=None, in_=nfr,
                in_offset=bass.IndirectOffsetOnAxis(ap=offs[:, 0:1], axis=0))
            xn3 = sb.tile([BL, NODE_DIM], bf16)
            nc.gpsimd.indirect_dma_start(
                out=xn3[:], out_offset=None, in_=nfr,
                in_offset=bass.IndirectOffsetOnAxis(ap=offs[:, 1:2], axis=0))

            # extended blended inputs: inp7 = [(1-z)*xg | 1-z | z*xd | z]
            # so that inp7 @ w7s == blended node/depot embedding (incl bias)
            isz2 = sb.tile([BL, 2], fp32)
            nc.vector.tensor_scalar(isz2[:], ids_c[:], 0, None,
                                    op0=Alu.is_equal)
            omz = sb.tile([BL, 2], fp32)
            nc.vector.tensor_scalar(omz[:], isz2[:], -1.0, 1.0,
                                    op0=Alu.mult, op1=Alu.add)
            c7 = sb.tile([BL, 7], bf16)
            nc.vector.tensor_scalar(c7[:, 0:3], xc3[:], omz[:, 0:1], None,
                                    op0=Alu.mult)
            nc.vector.tensor_copy(c7[:, 3:4], omz[:, 0:1])
            nc.vector.tensor_scalar(c7[:, 4:6], bp_c[:, 5:7], isz2[:, 0:1],
                                    None, op0=Alu.mult)
            nc.vector.tensor_copy(c7[:, 6:7], isz2[:, 0:1])
            n7 = sb.tile([BL, 7], bf16)
            nc.vector.tensor_scalar(n7[:, 0:3], xn3[:], omz[:, 1:2], None,
                                    op0=Alu.mult)
            nc.vector.tensor_copy(n7[:, 3:4], omz[:, 1:2])
            nc.vector.tensor_scalar(n7[:, 4:6], bp_c[:, 5:7], isz2[:, 1:2],
                                    None, op0=Alu.mult)
            nc.vector.tensor_copy(n7[:, 6:7], isz2[:, 1:2])
            st5b = sb.tile([BL, 5], bf16)
            nc.vector.tensor_copy(st5b[:], bp_c[:, 0:5])

            # transpose inputs to [feat, b]: one PSUM tile, three col blocks
            t3_p = ps.tile([7, 3 * BL], bf16, tag="pt")
            nc.tensor.transpose(t3_p[0:7, 0:BL], c7[:], identb[:])
            nc.tensor.transpose(t3_p[0:7, BL:2 * BL], n7[:], identb[:])
            nc.tensor.transpose(t3_p[0:5, 2 * BL:3 * BL], st5b[:], identb[:])
            t3in = sb.tile([7, 3 * BL], bf16)
            nc.vector.tensor_copy(t3in[0:7, 0:2 * BL], t3_p[0:7, 0:2 * BL])
            nc.vector.tensor_copy(t3in[0:5, 2 * BL:3 * BL],
                                  t3_p[0:5, 2 * BL:3 * BL])

            # ---------- embeddings (transposed [E, b]) ----------
            pemb = pse.tile([128, 3 * BL], fp32, tag="pemb")
            nc.tensor.matmul(pemb[:, 0:BL], lhsT=w7c, rhs=t3in[0:7, 0:BL],
                             start=True, stop=True)
            nc.tensor.matmul(pemb[:, BL:2 * BL], lhsT=w7c,
                             rhs=t3in[0:7, BL:2 * BL], start=True, stop=True)
            nc.tensor.matmul(pemb[:, 2 * BL:3 * BL], lhsT=ws5_c,
                             rhs=t3in[0:5, 2 * BL:3 * BL],
                             start=True, stop=True)
            t3 = sb.tile([128, 3 * BL], bf16)
            nc.vector.tensor_copy(t3[:], pemb[:])
            currT = t3[:, 0:BL]

            # ---------------- q, w37 ----------------
            qT_p = psq.tile([128, BL], fp32, tag="pqg")
            nc.tensor.matmul(qT_p[:], lhsT=wq_c[:, 0:EMB], rhs=t3[:, 0:BL],
                             start=True, stop=False)
            nc.tensor.matmul(qT_p[:], lhsT=wq_c[:, EMB:2 * EMB],
                             rhs=t3[:, BL:2 * BL], start=False, stop=False)
            nc.tensor.matmul(qT_p[:], lhsT=wq_c[:, 2 * EMB:3 * EMB],
                             rhs=t3[:, 2 * BL:3 * BL], start=False, stop=True)
            qT = sb.tile([128, BL], bf16)
            nc.vector.tensor_copy(qT[:], qT_p[:])

            # repeat q columns 4x: col p = j*BL + b  ->  q[:, b]
            qT4 = sb.tile([128, 128], bf16)
            nc.vector.tensor_copy(
                qT4[:].rearrange("p (j b) -> p j b", j=J),
                qT[:].unsqueeze(1).broadcast_to([128, J, BL]))

            # w37 [128,7]: 0:3 w3 | 3 cn | 4:6 wd | 6 cd   (per partition's b)
            w37_p = psq.tile([128, 7], fp32, tag="pqg")
            nc.tensor.matmul(w37_p[:], lhsT=qT4[:], rhs=w7kT_c,
                             start=True, stop=True)
            w37 = sb.tile([128, 7], fp32)
            nc.vector.tensor_copy(w37[:], w37_p[:])
            biasN = sb.tile([128, 1], fp32)
            nc.vector.tensor_scalar(biasN[:], w37[:, 3:4], NORM,
                                    MASK_BIG * NORM, op0=Alu.mult,
                                    op1=Alu.subtract)

            # ---------------- logits stream ----------------
            # L = sum_c x_c * w3_c  via chained per-partition-scalar ops
            x3 = x[:].rearrange("p (f c) -> p f c", c=3)
            L0 = sb.tile([128, NF], fp32)
            nc.vector.tensor_scalar(L0[:], x3[:, :, 0], w37[:, 0:1], None,
                                    op0=Alu.mult)
            L1 = sb.tile([128, NF], fp32)
            nc.vector.scalar_tensor_tensor(L1[:], x3[:, :, 1], w37[:, 1:2],
                                           L0[:], op0=Alu.mult, op1=Alu.add)
            L = sb.tile([128, NF], fp32)
            nc.vector.scalar_tensor_tensor(L[:], x3[:, :, 2], w37[:, 2:3],
                                           L1[:], op0=Alu.mult, op1=Alu.add)

            # depot logit at n=0 (rows 0:BL are the j=0 block, b-ordered):
            # L[b,0] = xd.wd + cd - cn  (cn re-added by the exp bias)
            cdmn = sb.tile([BL, 1], fp32)
            nc.vector.tensor_tensor(cdmn[:], w37[0:BL, 6:7], w37[0:BL, 3:4],
                                    op=Alu.subtract)
            dlm = sb.tile([BL, 2], fp32)
            nc.vector.tensor_tensor(dlm[:], bp_c[:, 5:7], w37[0:BL, 4:6],
                                    op=Alu.mult)
            dlr = sb.tile([BL, 1], fp32)
            nc.vector.tensor_reduce(dlr[:], dlm[:], axis=X, op=Alu.add)
            nc.vector.tensor_tensor(L[0:BL, 0:1], dlr[:], cdmn[:], op=Alu.add)

            # L += MASK_BIG*mask  (the -MASK_BIG constant lives in the exp bias)
            nc.vector.scalar_tensor_tensor(L[:], mku[:], MASK_BIG, L[:],
                                           op0=Alu.mult, op1=Alu.add)

            # E = exp(NORM*L + NORM*cn - MASK_BIG*NORM); accum -> row sums
            E = sb.tile([128, NF], bf16)
            sjact = sb.tile([128, 1], fp32)
            nc.scalar.activation(E[:], L[:], Act.Exp, bias=biasN[:],
                                 scale=NORM, accum_out=sjact[:])
            s3S = sb.tile([128, 4], fp32)
            nc.vector.tensor_copy(s3S[:, 3:4], sjact[:])

            # s3 partials: s3S[:,c] = sum_f E * x_c  (stt accumulate)
            sjunk = sb.tile([128, NF], bf16)
            for c in range(3):
                nc.vector.scalar_tensor_tensor(
                    sjunk[:], x3[:, :, c], 1.0, E[:], op0=Alu.mult,
                    op1=Alu.mult, accum_out=s3S[:, c:c + 1])

            # cross-j reduction: S[b,:] = sum_j s3S[j*BL+b,:]
            s3b_p = ps.tile([BL, 4], fp32, tag="pt")
            nc.tensor.matmul(s3b_p[:], lhsT=rep_eye[:], rhs=s3S[:],
                             start=True, stop=True)

            recipS = sb.tile([BL, 1], fp32)
            nc.vector.reciprocal(recipS[:], s3b_p[:, 3:4])

            # urhs [32,7]: 0:3 s3 | 3 1-a0 | 4:6 a0*xd | 6 a0
            urhs = sb.tile([BL, 7], bf16)
            a0 = sb.tile([BL, 1], fp32)
            nc.vector.tensor_scalar(a0[:], E[0:BL, 0:1], recipS[:],
                                    None, op0=Alu.mult)
            nc.vector.tensor_copy(urhs[:, 6:7], a0[:])
            t1 = sb.tile([BL, 3], fp32)
            nc.vector.tensor_scalar(t1[:], s3b_p[:, 0:3], recipS[:], None,
                                    op0=Alu.mult)
            t2 = sb.tile([BL, 3], fp32)
            nc.vector.tensor_scalar(t2[:], bp_c[:, 5:8], a0[:], None,
                                    op0=Alu.mult)
            nc.vector.tensor_tensor(urhs[:, 0:3], t1[:], t2[:],
                                    op=Alu.subtract)
            nc.vector.tensor_scalar(urhs[:, 3:4], a0[:], -1.0, 1.0,
                                    op0=Alu.mult, op1=Alu.add)
            nc.vector.tensor_copy(urhs[:, 4:6], t2[:, 0:2])

            urT_p = ps.tile([7, BL], bf16, tag="pt")
            nc.tensor.transpose(urT_p[:], urhs[:], identb[:])
            urT = sb.tile([7, BL], bf16)
            nc.vector.tensor_copy(urT[:], urT_p[:])

            # h [b, e] directly: curr_emb @ Wv1 + urhs @ W7v
            h_p = ps.tile([BL, EMB], fp32, tag="pt")
            nc.tensor.matmul(h_p[:], lhsT=currT, rhs=wv1_c,
                             start=True, stop=False)
            nc.tensor.matmul(h_p[:], lhsT=urT[:], rhs=w7v_c,
                             start=False, stop=True)
            h_sb = sb.tile([BL, EMB], fp32)
            nc.vector.tensor_copy(h_sb[:], h_p[:])
            nc.sync.dma_start(out[:], h_sb[:])

    if finalize:
        nc.finalize()
    return nc


def _shard_inputs(node_feats, state, W_node, b_node, W_depot, b_depot,
                  W_state, b_state, w_q, w_k, w_v, curr_node_id,
                  next_node_id, mask):
    import ml_dtypes

    f32 = np.float32
    bf = ml_dtypes.bfloat16
    node_feats = np.ascontiguousarray(node_feats, dtype=f32)
    nf_bf16 = node_feats.astype(bf)
    state = np.ascontiguousarray(state, dtype=f32)
    mask_u8 = np.ascontiguousarray(mask).astype(np.uint8)
    ids = np.stack([np.asarray(curr_node_id), np.asarray(next_node_id)],
                   axis=1).astype(np.int32)

    W_node = np.asarray(W_node, dtype=f32)
    b_node = np.asarray(b_node, dtype=f32)
    W_depot = np.asarray(W_depot, dtype=f32)
    b_depot = np.asarray(b_depot, dtype=f32)
    W_state = np.asarray(W_state, dtype=f32)
    b_state = np.asarray(b_state, dtype=f32)
    w_q = np.asarray(w_q, dtype=f32)
    w_k = np.asarray(w_k, dtype=f32)
    w_v = np.asarray(w_v, dtype=f32)

    w7s = np.concatenate([W_node, b_node[None, :], W_depot,
                          b_depot[None, :]], axis=0)           # [7,128]
    W7k = w7s @ w_k[128:256]                                   # [7,128]
    W7v = w7s @ w_v[128:256]                                   # [7,128]
    wpack = np.zeros((7, 384), f32)
    wpack[:, 0:128] = w7s
    wpack[0:4, 128:256] = W_state
    wpack[4, 128:256] = b_state
    wpack[:, 256:384] = W7v
    wbig = np.zeros((128, WBIG_COLS), f32)
    wbig[:, 0:128] = w_q[0:128]
    wbig[:, 128:256] = w_q[128:256]
    wbig[:, 256:384] = w_q[256:384]
    wbig[:, 384:512] = w_v[0:128]
    wbig[:, 512:519] = W7k.T

    shared = {
        "wpack": np.ascontiguousarray(wpack.astype(bf)),
        "wbig": np.ascontiguousarray(wbig.astype(bf)),
    }
    in_maps = []
    for i in range(NCORES):
        s = slice(i * BL, (i + 1) * BL)
        m = dict(shared)
        m["node_feats"] = np.ascontiguousarray(nf_bf16[s])
        bp = np.concatenate([state[s], np.ones((BL, 1), f32),
                             node_feats[s, 0, :]], axis=1)
        m["bpack"] = np.ascontiguousarray(bp)
        m["ids"] = np.ascontiguousarray(ids[s])
        m["mask_u8"] = np.ascontiguousarray(mask_u8[s])
        in_maps.append(m)
    return in_maps


def _run(inputs, trace=False):
    from concourse.bass_utils import run_bass_kernel_spmd

    if "nc" not in _CACHE:
        _CACHE["nc"] = _build()
    nc = _CACHE["nc"]
    in_maps = _shard_inputs(**inputs)
    res = run_bass_kernel_spmd(nc, in_maps, core_ids=list(range(NCORES)),
                               trace=trace)
    full = np.concatenate([r["out"] for r in res.results], axis=0)
    return full, res


def kernel(**inputs):
    full, _ = _run(inputs, trace=False)
    return full


# revision 34
# speedup vs baseline: 1.8472x; 1.1376x over previous
"""Trainium2 Bass kernel for nn_AttentionGraphEncoder (gnn_message_passing).

Math restructure (exact, not approximate):
  Per batch b the reference computes masked attention over N=2048 nodes
  whose keys/values are affine in the raw 3-dim node coordinates, so
    logits[n] = x[n] . w3(b) + const,   w3(b) = (w7s @ Wk2) rows @ q(b)
    h = curr_emb @ Wv1 + urhs(b) @ (w7s @ Wv2)
  with urhs = [s3 | 1-a0 | a0*xd | a0], s3 = sum_{n>=1} attn[n] x[n].
  The big [N,2E]@[2E,E] matmuls disappear; the kernel streams node_feats
  (bf16) once through the DVE and does tiny matmuls.  Weight-only
  products (w7s@Wk2, w7s@Wv2) are folded on the host.

Sharding: pure data parallel, batch 256 -> 32 per core across 8 cores.

Perf notes:
  - DMA issue blocks the issuing engine queue, so the x stream is split
    across the Scalar and Sync queues; ids goes first (it heads the
    longest chain: gather -> q -> w3 -> logits).
  - Node/depot blending happens in the gathered input vector (inp7 =
    [(1-z)xg | 1-z | z*xd | z]) so one [7,128] matmul produces the
    blended embedding including biases.
  - All small matmuls run bf16 (fp32 PE mode is 2-pass); logits/sums
    accumulate f32.
  - L and s3 use per-partition-scalar chained ops (scalar_tensor_tensor
    with f32 accumulate) instead of materialize+reduce.
"""

import math

import numpy as np

B, N, NODE_DIM, STATE_DIM, EMB = 256, 2048, 3, 4, 128
NCORES = 8
BL = B // NCORES          # 32 batch elements per core
J = 4                     # node-chunks per batch -> 128 partitions (j*BL + b)
NF = N // J               # 512 nodes per partition row
KC = 4                    # free-dim DMA chunks of the x stream
NFC = NF // KC
NORM = 1.0 / math.sqrt(EMB)
MASK_BIG = 400.0          # pre-NORM additive mask magnitude (400*NORM ~ 35)
WBIG_COLS = 3 * EMB + EMB + 8             # wq | Wv1 | W7kT | pad = 520

_CACHE = {}


def _build(finalize=True):
    import concourse.bacc as bacc
    import concourse.bass as bass
    import concourse.mybir as mybir
    import concourse.tile as tile
    from concourse.masks import make_identity

    fp32 = mybir.dt.float32
    bf16 = mybir.dt.bfloat16
    i32 = mybir.dt.int32
    u8 = mybir.dt.uint8
    Alu = mybir.AluOpType
    Act = mybir.ActivationFunctionType
    X = mybir.AxisListType.X

    nc = bacc.Bacc("TRN2")

    nfd = nc.dram_tensor("node_feats", [BL, N, NODE_DIM], bf16,
                         kind="ExternalInput")
    # bpack: state(4) | 1.0 | x0(3) | ids(2, int32 bitcast)  per row
    bpk = nc.dram_tensor("bpack", [BL, 10], fp32, kind="ExternalInput")
    # wpack [7, 384] bf16: w7s | [W_state; b_state] (rows 0:5) | W7v
    wpk = nc.dram_tensor("wpack", [7, 384], bf16, kind="ExternalInput")
    # wbig [128, 520] bf16: wq(384) | Wv1(128) | W7kT(7) | pad
    wbg = nc.dram_tensor("wbig", [128, WBIG_COLS], bf16, kind="ExternalInput")
    mk = nc.dram_tensor("mask_u8", [BL, N], u8, kind="ExternalInput")
    out = nc.dram_tensor("out", [BL, EMB], fp32, kind="ExternalOutput")

    with tile.TileContext(nc, pool_alloc_mode="queue") as tc:
        with (
            tc.tile_pool(name="sb", bufs=1) as sb,
            tc.tile_pool(name="ps", bufs=3, space="PSUM") as ps,
            tc.tile_pool(name="pse", bufs=1, space="PSUM") as pse,
            tc.tile_pool(name="psq", bufs=2, space="PSUM") as psq,
        ):
            # iota first: the gather offsets need it immediately
            iota_p = sb.tile([BL, 1], i32)
            nc.gpsimd.iota(iota_p[:], pattern=[[0, 1]], base=0,
                           channel_multiplier=N)

            # ------------------- input DMAs -------------------
            bp_sb = sb.tile([BL, 10], fp32)
            nc.sync.dma_start(bp_sb[:], bpk[:])
            ids_c = bp_sb[:, 8:10].bitcast(i32)
            bp_c = bp_sb
            # ---------------- gathers ----------------
            offs = sb.tile([BL, 2], i32)
            nc.vector.tensor_tensor(offs[:], ids_c,
                                    iota_p[:].to_broadcast([BL, 2]),
                                    op=Alu.add)
            nfr = nfd[:].rearrange("b n c -> (b n) c")
            xc3 = sb.tile([BL, NODE_DIM], bf16)
            nc.gpsimd.indirect_dma_start(
                out=xc3[:], out_offset=None, in_=nfr,
                in_offset=bass.IndirectOffsetOnAxis(ap=offs[:, 0:1], axis=0))
            xn3 = sb.tile([BL, NODE_DIM], bf16)
            nc.gpsimd.indirect_dma_start(
                out=xn3[:], out_offset=None, in_=nfr,
                in_offset=bass.IndirectOffsetOnAxis(ap=offs[:, 1:2], axis=0))

            wp_sb = sb.tile([7, 384], bf16)
            nc.sync.dma_start(wp_sb[:], wpk[:])
            wb_sb = sb.tile([128, WBIG_COLS], bf16)
            nc.sync.dma_start(wb_sb[:], wbg[:])
            x = sb.tile([128, NF * 3], bf16)
            xview = (nfd[:].rearrange("b (j f) c -> b j (f c)", j=J)
                     .transpose([1, 0, 2]))                 # [J, BL, NF*3]
            for k in range(KC):
                ksl = slice(k * NFC * 3, (k + 1) * NFC * 3)
                eng = nc.scalar if k % 2 == 0 else nc.sync
                eng.dma_start(x[:, ksl], xview[:, :, ksl])
            mku = sb.tile([128, NF], u8)
            mview = (mk[:].rearrange("b (j f) -> b j f", j=J)
                     .transpose([1, 0, 2]))                 # [J, BL, NF]
            nc.scalar.dma_start(mku[:], mview[:])

            # remaining gpsimd constants (run during the gather transfers)
            identb = sb.tile([BL, BL], bf16)
            make_identity(nc, identb[:])
            # rep_eye[p, y] = 1 iff p % BL == y  (cross-j reduce as a matmul)
            rep_eye = sb.tile([128, BL], fp32)
            nc.gpsimd.memset(rep_eye[:], 0.0)
            for j in range(J):
                nc.gpsimd.affine_select(
                    out=rep_eye[:], in_=rep_eye[:],
                    compare_op=Alu.not_equal, fill=1.0,
                    base=-BL * j, pattern=[[-1, BL]], channel_multiplier=1)
            # PE warm-up: one op depending on the LAST gpsimd constant so
            # later PE ops see all Pool ticks as observed.
            junk_p = ps.tile([1, 1], fp32, tag="pt")
            nc.tensor.matmul(junk_p[:], lhsT=rep_eye[:, 0:1],
                             rhs=rep_eye[:, 0:1], start=True, stop=True)

            w7c = wp_sb[:, 0:128]          # [7,128]  [Wn; bn; Wd; bd]
            ws5_c = wp_sb[0:5, 128:256]    # [5,128]  [Ws; bs]
            w7v_c = wp_sb[:, 256:384]      # [7,128]  w7s @ Wv2
            wq_c = wb_sb[:, 0:3 * EMB]
            wv1_c = wb_sb[:, 3 * EMB:4 * EMB]
            w7kT_c = wb_sb[:, 4 * EMB:4 * EMB + 7]   # [128,7]

            # extended blended inputs: inp7 = [(1-z)*xg | 1-z | z*xd | z]
            # so that inp7 @ w7s == blended node/depot embedding (incl bias)
            isz2 = sb.tile([BL, 2], fp32)
            nc.vector.tensor_scalar(isz2[:], ids_c, 0, None,
                                    op0=Alu.is_equal)
            omz = sb.tile([BL, 2], fp32)
            nc.vector.tensor_scalar(omz[:], isz2[:], -1.0, 1.0,
                                    op0=Alu.mult, op1=Alu.add)
            c7 = sb.tile([BL, 7], bf16)
            nc.vector.tensor_scalar(c7[:, 0:3], xc3[:], omz[:, 0:1], None,
                                    op0=Alu.mult)
            nc.vector.tensor_copy(c7[:, 3:4], omz[:, 0:1])
            nc.vector.tensor_scalar(c7[:, 4:6], bp_c[:, 5:7], isz2[:, 0:1],
                                    None, op0=Alu.mult)
            nc.vector.tensor_copy(c7[:, 6:7], isz2[:, 0:1])
            n7 = sb.tile([BL, 7], bf16)
            nc.vector.tensor_scalar(n7[:, 0:3], xn3[:], omz[:, 1:2], None,
                                    op0=Alu.mult)
            nc.vector.tensor_copy(n7[:, 3:4], omz[:, 1:2])
            nc.vector.tensor_scalar(n7[:, 4:6], bp_c[:, 5:7], isz2[:, 1:2],
                                    None, op0=Alu.mult)
            nc.vector.tensor_copy(n7[:, 6:7], isz2[:, 1:2])
            st5b = sb.tile([BL, 5], bf16)
            nc.vector.tensor_copy(st5b[:], bp_c[:, 0:5])

            # transpose inputs to [feat, b]: one PSUM tile, three col blocks
            t3_p = ps.tile([7, 3 * BL], bf16, tag="pt")
            nc.tensor.transpose(t3_p[0:7, 0:BL], c7[:], identb[:])
            nc.tensor.transpose(t3_p[0:7, BL:2 * BL], n7[:], identb[:])
            nc.tensor.transpose(t3_p[0:5, 2 * BL:3 * BL], st5b[:], identb[:])
            t3in = sb.tile([7, 3 * BL], bf16)
            nc.vector.tensor_copy(t3in[0:7, 0:2 * BL], t3_p[0:7, 0:2 * BL])
            nc.vector.tensor_copy(t3in[0:5, 2 * BL:3 * BL],
                                  t3_p[0:5, 2 * BL:3 * BL])

            # ---------- embeddings (transposed [E, b]) ----------
            pemb = pse.tile([128, 3 * BL], fp32, tag="pemb")
            nc.tensor.matmul(pemb[:, 0:BL], lhsT=w7c, rhs=t3in[0:7, 0:BL],
                             start=True, stop=True)
            nc.tensor.matmul(pemb[:, BL:2 * BL], lhsT=w7c,
                             rhs=t3in[0:7, BL:2 * BL], start=True, stop=True)
            nc.tensor.matmul(pemb[:, 2 * BL:3 * BL], lhsT=ws5_c,
                             rhs=t3in[0:5, 2 * BL:3 * BL],
                             start=True, stop=True)
            t3 = sb.tile([128, 3 * BL], bf16)
            nc.vector.tensor_copy(t3[:], pemb[:])
            currT = t3[:, 0:BL]

            # ---------------- q, w37 ----------------
            qT_p = psq.tile([128, BL], fp32, tag="pqg")
            nc.tensor.matmul(qT_p[:], lhsT=wq_c[:, 0:EMB], rhs=t3[:, 0:BL],
                             start=True, stop=False)
            nc.tensor.matmul(qT_p[:], lhsT=wq_c[:, EMB:2 * EMB],
                             rhs=t3[:, BL:2 * BL], start=False, stop=False)
            nc.tensor.matmul(qT_p[:], lhsT=wq_c[:, 2 * EMB:3 * EMB],
                             rhs=t3[:, 2 * BL:3 * BL], start=False, stop=True)
            qT = sb.tile([128, BL], bf16)
            nc.vector.tensor_copy(qT[:], qT_p[:])

            # repeat q columns 4x: col p = j*BL + b  ->  q[:, b]
            qT4 = sb.tile([128, 128], bf16)
            nc.vector.tensor_copy(
                qT4[:].rearrange("p (j b) -> p j b", j=J),
                qT[:].unsqueeze(1).broadcast_to([128, J, BL]))

            # w37 [128,7]: 0:3 w3 | 3 cn | 4:6 wd | 6 cd   (per partition's b)
            w37_p = psq.tile([128, 7], fp32, tag="pqg")
            nc.tensor.matmul(w37_p[:], lhsT=qT4[:], rhs=w7kT_c,
                             start=True, stop=True)
            w37 = sb.tile([128, 7], fp32)
            nc.vector.tensor_copy(w37[:], w37_p[:])
            biasN = sb.tile([128, 1], fp32)
            nc.vector.tensor_scalar(biasN[:], w37[:, 3:4], NORM,
                                    MASK_BIG * NORM, op0=Alu.mult,
                                    op1=Alu.subtract)

            # ---------------- logits stream ----------------
            # L = sum_c x_c * w3_c  via chained per-partition-scalar ops
            x3 = x[:].rearrange("p (f c) -> p f c", c=3)
            L0 = sb.tile([128, NF], fp32)
            nc.vector.tensor_scalar(L0[:], x3[:, :, 0], w37[:, 0:1], None,
                                    op0=Alu.mult)
            L1 = sb.tile([128, NF], fp32)
            nc.vector.scalar_tensor_tensor(L1[:], x3[:, :, 1], w37[:, 1:2],
                                           L0[:], op0=Alu.mult, op1=Alu.add)
            L = sb.tile([128, NF], fp32)
            nc.vector.scalar_tensor_tensor(L[:], x3[:, :, 2], w37[:, 2:3],
                                           L1[:], op0=Alu.mult, op1=Alu.add)

            # depot logit at n=0 (rows 0:BL are the j=0 block, b-ordered):
            # L[b,0] = xd.wd + cd - cn  (cn re-added by the exp bias)
            cdmn = sb.tile([BL, 1], fp32)
            nc.vector.tensor_tensor(cdmn[:], w37[0:BL, 6:7], w37[0:BL, 3:4],
                                    op=Alu.subtract)
            dlm = sb.tile([BL, 2], fp32)
            nc.vector.tensor_tensor(dlm[:], bp_c[:, 5:7], w37[0:BL, 4:6],
                                    op=Alu.mult)
            dlr = sb.tile([BL, 1], fp32)
            nc.vector.tensor_reduce(dlr[:], dlm[:], axis=X, op=Alu.add)
            nc.vector.tensor_tensor(L[0:BL, 0:1], dlr[:], cdmn[:], op=Alu.add)

            # L += MASK_BIG*mask  (the -MASK_BIG constant lives in the exp bias)
            nc.vector.scalar_tensor_tensor(L[:], mku[:], MASK_BIG, L[:],
                                           op0=Alu.mult, op1=Alu.add)

            # E = exp(NORM*L + NORM*cn - MASK_BIG*NORM); accum -> row sums
            E = sb.tile([128, NF], bf16)
            sjact = sb.tile([128, 1], fp32)
            nc.scalar.activation(E[:], L[:], Act.Exp, bias=biasN[:],
                                 scale=NORM, accum_out=sjact[:])
            s3S = sb.tile([128, 4], fp32)
            nc.vector.tensor_copy(s3S[:, 3:4], sjact[:])

            # s3 partials: s3S[:,c] = sum_f E * x_c  (stt accumulate)
            sjunk = sb.tile([128, NF], bf16)
            for c in range(3):
                nc.vector.scalar_tensor_tensor(
                    sjunk[:], x3[:, :, c], 1.0, E[:], op0=Alu.mult,
                    op1=Alu.mult, accum_out=s3S[:, c:c + 1])

            # cross-j reduction: S[b,:] = sum_j s3S[j*BL+b,:]
            s3b_p = ps.tile([BL, 4], fp32, tag="pt")
            nc.tensor.matmul(s3b_p[:], lhsT=rep_eye[:], rhs=s3S[:],
                             start=True, stop=True)

            recipS = sb.tile([BL, 1], fp32)
            nc.vector.reciprocal(recipS[:], s3b_p[:, 3:4])

            # urhs [32,7]: 0:3 s3 | 3 1-a0 | 4:6 a0*xd | 6 a0
            urhs = sb.tile([BL, 7], bf16)
            a0 = sb.tile([BL, 1], fp32)
            nc.vector.tensor_scalar(a0[:], E[0:BL, 0:1], recipS[:],
                                    None, op0=Alu.mult)
            nc.vector.tensor_copy(urhs[:, 6:7], a0[:])
            t1 = sb.tile([BL, 3], fp32)
            nc.vector.tensor_scalar(t1[:], s3b_p[:, 0:3], recipS[:], None,
                                    op0=Alu.mult)
            t2 = sb.tile([BL, 3], fp32)
            nc.vector.tensor_scalar(t2[:], bp_c[:, 5:8], a0[:], None,
                                    op0=Alu.mult)
            nc.vector.tensor_tensor(urhs[:, 0:3], t1[:], t2[:],
                                    op=Alu.subtract)
            nc.vector.tensor_scalar(urhs[:, 3:4], a0[:], -1.0, 1.0,
                                    op0=Alu.mult, op1=Alu.add)
            nc.vector.tensor_copy(urhs[:, 4:6], t2[:, 0:2])

            urT_p = ps.tile([7, BL], bf16, tag="pt")
            nc.tensor.transpose(urT_p[:], urhs[:], identb[:])
            urT = sb.tile([7, BL], bf16)
            nc.vector.tensor_copy(urT[:], urT_p[:])

            # h [b, e] directly: curr_emb @ Wv1 + urhs @ W7v
            h_p = ps.tile([BL, EMB], fp32, tag="pt")
            nc.tensor.matmul(h_p[:], lhsT=currT, rhs=wv1_c,
                             start=True, stop=False)
            nc.tensor.matmul(h_p[:], lhsT=urT[:], rhs=w7v_c,
                             start=False, stop=True)
            h_sb = sb.tile([BL, EMB], fp32)
            nc.vector.tensor_copy(h_sb[:], h_p[:])
            nc.sync.dma_start(out[:], h_sb[:])

    if finalize:
        nc.finalize()
    return nc


def _shard_inputs(node_feats, state, W_node, b_node, W_depot, b_depot,
                  W_state, b_state, w_q, w_k, w_v, curr_node_id,
                  next_node_id, mask):
    import ml_dtypes

    f32 = np.float32
    bf = ml_dtypes.bfloat16
    node_feats = np.ascontiguousarray(node_feats, dtype=f32)
    nf_bf16 = node_feats.astype(bf)
    state = np.ascontiguousarray(state, dtype=f32)
    mask_u8 = np.ascontiguousarray(mask).astype(np.uint8)
    ids = np.stack([np.asarray(curr_node_id), np.asarray(next_node_id)],
                   axis=1).astype(np.int32)

    W_node = np.asarray(W_node, dtype=f32)
    b_node = np.asarray(b_node, dtype=f32)
    W_depot = np.asarray(W_depot, dtype=f32)
    b_depot = np.asarray(b_depot, dtype=f32)
    W_state = np.asarray(W_state, dtype=f32)
    b_state = np.asarray(b_state, dtype=f32)
    w_q = np.asarray(w_q, dtype=f32)
    w_k = np.asarray(w_k, dtype=f32)
    w_v = np.asarray(w_v, dtype=f32)

    w7s = np.concatenate([W_node, b_node[None, :], W_depot,
                          b_depot[None, :]], axis=0)           # [7,128]
    W7k = w7s @ w_k[128:256]                                   # [7,128]
    W7v = w7s @ w_v[128:256]                                   # [7,128]
    wpack = np.zeros((7, 384), f32)
    wpack[:, 0:128] = w7s
    wpack[0:4, 128:256] = W_state
    wpack[4, 128:256] = b_state
    wpack[:, 256:384] = W7v
    wbig = np.zeros((128, WBIG_COLS), f32)
    wbig[:, 0:128] = w_q[0:128]
    wbig[:, 128:256] = w_q[128:256]
    wbig[:, 256:384] = w_q[256:384]
    wbig[:, 384:512] = w_v[0:128]
    wbig[:, 512:519] = W7k.T

    shared = {
        "wpack": np.ascontiguousarray(wpack.astype(bf)),
        "wbig": np.ascontiguousarray(wbig.astype(bf)),
    }
    in_maps = []
    for i in range(NCORES):
        s = slice(i * BL, (i + 1) * BL)
        m = dict(shared)
        m["node_feats"] = np.ascontiguousarray(nf_bf16[s])
        bp = np.concatenate([state[s], np.ones((BL, 1), f32),
                             node_feats[s, 0, :],
                             ids[s].view(f32)], axis=1)
        m["bpack"] = np.ascontiguousarray(bp)
        m["mask_u8"] = np.ascontiguousarray(mask_u8[s])
        in_maps.append(m)
    return in_maps


def _run(inputs, trace=False):
    from concourse.bass_utils import run_bass_kernel_spmd

    if "nc" not in _CACHE:
        _CACHE["nc"] = _build()
    nc = _CACHE["nc"]
    in_maps = _shard_inputs(**inputs)
    res = run_bass_kernel_spmd(nc, in_maps, core_ids=list(range(NCORES)),
                               trace=trace)
    full = np.concatenate([r["out"] for r in res.results], axis=0)
    return full, res


def kernel(**inputs):
    full, _ = _run(inputs, trace=False)
    return full
